# revision 2
# baseline (speedup 1.0000x reference)
"""Grouped-Query Attention on 8 Trainium2 NeuronCores — v2.

Sharding: TP-4 over KV groups x DP-2 over batch.
Core c handles batch b = c // 4, group g = c % 4 (4 query heads, 1 KV group).

Differences vs v1:
  - V is projected directly into natural [t, d] layout in phase 1
    (stationary = xt tile, moving = Wv chunk) -- no PE transposes.
  - Softmax denominators: E accumulated over tk on DVE (bf16 2x), then a
    single allones[128,128] @ accE matmul broadcasts the denominator --
    removes 160 sum-matmuls + 16 broadcast-matmuls from PE.
  - Causal masking via Pool-engine affine_select (in-place on E).
  - Software-pipelined PE emission: S(h) pairs interleave with PV(h-1)
    pairs and O-projection(qc-1) filler matmuls.
  - DMA: wq split per head and interleaved with the xt stream; weights
    pre-transposed on host for 4KB-contiguous descriptors.
"""

import numpy as np
import ml_dtypes

EMBED = 2048
T = 2048
D = 128           # head dim
NQH = 16          # query heads
NG = 4            # kv groups
HPG = NQH // NG   # query heads per group = 4
NCORES = 8
ECH = EMBED // 128   # 16 contraction chunks
TC = T // 512        # 4 t-chunks of 512
TT = T // 128        # 16 t-tiles of 128
SCALE = 1.0 / float(np.sqrt(D))

_PROG = {}


def build_program():
    if "nc" in _PROG:
        return _PROG["nc"]

    from contextlib import ExitStack
    import concourse.mybir as mybir
    from concourse import bacc, tile

    # Drop redundant consecutive Ldweights with identical keys (the Tile
    # legalizer emits one per Matmult even when the stationary is unchanged).
    if not getattr(tile.tile_legalize, "_ldw_dedup", False):
        _orig_legalize = tile.tile_legalize

        def _dedup_legalize(ordered, nc_):
            ordered = _orig_legalize(ordered, nc_)
            for bb, insts in ordered.items():
                out = []
                state = None
                for inst in insts:
                    tn = type(inst).__name__
                    if tn == "InstLdweights":
                        key = (
                            str(inst.ins[0]),
                            str(getattr(inst, "is_transpose", None)),
                            str(getattr(inst, "tile_position", None)),
                            str(getattr(inst, "perf_mode", None)),
                        )
                        if key == state:
                            continue
                        state = key
                    out.append(inst)
                ordered[bb] = out
            return ordered

        _dedup_legalize._ldw_dedup = True
        tile.tile_legalize = _dedup_legalize

    dt = mybir.dt
    BF = dt.bfloat16
    F32 = dt.float32
    AF = mybir.ActivationFunctionType

    nc = bacc.Bacc("TRN2", target_bir_lowering=False, debug=False)

    xt_d = nc.dram_tensor("xt", [ECH, 128, T], BF, kind="ExternalInput")
    wq_d = nc.dram_tensor("wq", [HPG, 128, ECH * D], BF, kind="ExternalInput")
    wk_d = nc.dram_tensor("wk", [128, ECH * D], BF, kind="ExternalInput")
    wv_d = nc.dram_tensor("wv", [128, ECH * D], BF, kind="ExternalInput")
    wo_d = nc.dram_tensor("wo", [HPG, 128, EMBED], BF, kind="ExternalInput")
    bq_d = nc.dram_tensor("bq", [128, HPG], F32, kind="ExternalInput")
    bk_d = nc.dram_tensor("bk", [128, 1], F32, kind="ExternalInput")
    bv_d = nc.dram_tensor("bv", [1, 128], BF, kind="ExternalInput")
    out_d = nc.dram_tensor("out", [T, EMBED], BF, kind="ExternalOutput")

    with tile.TileContext(nc) as tc, ExitStack() as ctx:
        pers = ctx.enter_context(tc.tile_pool(name="pers", bufs=1))

        wq_sb = pers.tile([128, HPG, ECH * D], BF)
        wk_sb = pers.tile([128, ECH * D], BF)
        wv_sb = pers.tile([128, ECH * D], BF)
        wo_sb = pers.tile([128, HPG, EMBED], BF)
        bq_sb = pers.tile([128, HPG], F32)
        bk_sb = pers.tile([128, 1], F32)
        bv_sb = pers.tile([1, 128], BF)
        qT_sb = pers.tile([128, HPG, T], BF)
        kT_sb = pers.tile([128, T], BF)
        v_sb = pers.tile([128, TT, D], BF)
        nT_a = pers.tile([128, HPG, 512], BF)
        nT_b = pers.tile([128, HPG, 512], BF)
        ones1 = pers.tile([1, 128], BF)
        allones = pers.tile([128, 128], BF)

        nc.gpsimd.memset(ones1[:], 1.0)
        nc.gpsimd.memset(allones[:], 1.0)

        # ---- Phase 1: projections ----
        if True:
            xt_sb = pers.tile([128, ECH, T], BF)

            # All input DMAs on the sync queue in priority order (the DMA
            # engines are a shared serial resource): wk/wv + biases first,
            # then xt chunks with wq heads interleaved, wo last (needed only
            # by the O-projection ~60us later).
            nc.sync.dma_start(wk_sb[:, 0:D], wk_d[:, 0:D])
            nc.sync.dma_start(xt_sb[:, 0, 0:1024], xt_d[0][:, 0:1024])
            nc.sync.dma_start(wv_sb[:, 0:D], wv_d[:, 0:D])
            nc.sync.dma_start(xt_sb[:, 0, 1024:], xt_d[0][:, 1024:])
            nc.sync.dma_start(wk_sb[:, D:], wk_d[:, D:])
            nc.sync.dma_start(xt_sb[:, 1, :], xt_d[1])
            nc.sync.dma_start(wv_sb[:, D:], wv_d[:, D:])
            nc.sync.dma_start(xt_sb[:, 2, :], xt_d[2])
            nc.sync.dma_start(bk_sb[:], bk_d[:])
            nc.sync.dma_start(bv_sb[:], bv_d[:])
            nc.sync.dma_start(bq_sb[:], bq_d[:])
            wq_next = 0
            for ec in range(3, ECH):
                nc.sync.dma_start(xt_sb[:, ec, :], xt_d[ec])
                if ec % 6 == 2 and wq_next < HPG:
                    nc.sync.dma_start(wq_sb[:, wq_next, :], wq_d[wq_next])
                    wq_next += 1
            while wq_next < HPG:
                nc.sync.dma_start(wq_sb[:, wq_next, :], wq_d[wq_next])
                wq_next += 1
            nc.sync.dma_start(wo_sb[:], wo_d.ap().rearrange("h p e -> p h e"))

            # --- phase-1 PSUM: v accumulates on the right stack, kT on
            # the left.  kT's pool releases into ring1 (q half-passes + s2
            # pairs); v's pool releases into the po/pf pools.  No pool
            # barrier separates phase 1 from attention.
            kvB = tc.alloc_tile_pool(name="kvB", bufs=1, space="PSUM",
                                     side="right")
            vps = kvB.tile([128, TT, D], F32, tag="v", name="vps")
            kvA = tc.alloc_tile_pool(name="kvA", bufs=1, space="PSUM",
                                     side="left")
            kps = kvA.tile([128, T], F32, tag="k", name="kps")
            for ec in range(ECH):
                wkc = wk_sb[:, ec * D:(ec + 1) * D]
                for t5 in range(TC):
                    nc.tensor.matmul(
                        kps[:, t5 * 512:(t5 + 1) * 512],
                        wkc,
                        xt_sb[:, ec, t5 * 512:(t5 + 1) * 512],
                        start=(ec == 0),
                        stop=(ec == ECH - 1),
                    )
                wvc = wv_sb[:, ec * D:(ec + 1) * D]
                for tt in range(TT):
                    # 4 tt-tiles share a PSUM bank; `start` zeroes the whole
                    # 2KB bank region, so only the bank's first write starts
                    # the group and its last write stops it.
                    nc.tensor.matmul(
                        vps[:, tt, :],
                        xt_sb[:, ec, tt * D:(tt + 1) * D],
                        wvc,
                        start=(ec == 0 and tt % 4 == 0),
                        stop=False,
                    )
            # fold bv in as a rank-1 accumulation, closing each group
            for tt in range(TT):
                nc.tensor.matmul(
                    vps[:, tt, :], ones1[:], bv_sb[:],
                    start=False, stop=(tt % 4 == 3),
                )
            # drains: kT spread over ACT/DVE/Pool; v split DVE/Pool
            for t5 in range(TC):
                sl = slice(t5 * 512, (t5 + 1) * 512)
                if t5 in (1, 2):
                    nc.vector.tensor_scalar_add(
                        kT_sb[:, sl], kps[:, sl], bk_sb[:]
                    )
                else:
                    nc.scalar.activation(
                        kT_sb[:, sl], kps[:, sl], AF.Identity, bias=bk_sb[:]
                    )
            for tt in range(TT):
                if tt % 2 == 0:
                    nc.vector.tensor_copy(v_sb[:, tt, :], vps[:, tt, :])
                else:
                    nc.scalar.activation(
                        v_sb[:, tt, :], vps[:, tt, :], AF.Identity
                    )
            kvA.release()
            ring1 = tc.alloc_tile_pool(name="ring1", bufs=2, space="PSUM",
                                       side="left")
            kvB.release()
            psoP = tc.alloc_tile_pool(name="psoP", bufs=2, space="PSUM",
                                      side="right")
            psfP = tc.alloc_tile_pool(name="psfP", bufs=2, space="PSUM",
                                      side="right")

            def q_half(j, th):
                ps = ring1.tile([128, 1024], F32, tag="r", name=f"q{j}h{th}")
                for ec in range(ECH):
                    lhsT = wq_sb[:, j, ec * D:(ec + 1) * D]
                    for t5 in range(2):
                        lo = th * 1024 + t5 * 512
                        nc.tensor.matmul(
                            ps[:, t5 * 512:(t5 + 1) * 512],
                            lhsT,
                            xt_sb[:, ec, lo:lo + 512],
                            start=(ec == 0),
                            stop=(ec == ECH - 1),
                        )
                if th == 0:
                    nc.scalar.activation(
                        qT_sb[:, j, th * 1024:(th + 1) * 1024], ps[:],
                        AF.Identity, bias=bq_sb[:, j:j + 1],
                    )
                else:
                    nc.vector.tensor_scalar_add(
                        qT_sb[:, j, th * 1024:(th + 1) * 1024], ps[:],
                        bq_sb[:, j:j + 1],
                    )

        # ---- Phase 2: attention + O-projection, software-pipelined ----
        with (
            tc.tile_pool(name="eb", bufs=2) as ebp,
            tc.tile_pool(name="acp", bufs=2) as acp,
            tc.tile_pool(name="rcp", bufs=2) as rcp,
            tc.tile_pool(name="fsb", bufs=4) as fsb,
        ):
            nT_for = {}

            def dg_off(qc, tk):
                # columns [0, off) of tile tk are fully below the causal
                # diagonal (masked out) -- skip computing them entirely
                return max(0, (tk - 4 * qc)) * D if tk >= 4 * qc else 0

            def s_pair(h, qc, tkp, nk, E, accE):
                """Two S matmuls -> exp pair -> mask diag -> accumulate."""
                s2 = ring1.tile([128, 2, 512], F32, tag="r", name="s2t")
                # both tiles of a pair computed at the pair's min diagonal
                # offset so the exp reads a fully-written region
                woff = dg_off(qc, tkp * 2)
                for u in range(2):
                    tk = tkp * 2 + u
                    nc.tensor.matmul(
                        s2[:, u, woff:],
                        kT_sb[:, tk * D:(tk + 1) * D],
                        qT_sb[:, h, qc * 512 + woff:(qc + 1) * 512],
                        start=True,
                        stop=True,
                    )
                nc.scalar.activation(
                    E[:, tkp * 2:tkp * 2 + 2, woff:], s2[:, :, woff:],
                    AF.Exp, scale=SCALE
                )
                for u in range(2):
                    tk = tkp * 2 + u
                    off = dg_off(qc, tk)
                    if tk >= 4 * qc:
                        # zero E[p, woff+f] where woff + f < off + p
                        nc.gpsimd.affine_select(
                            out=E[:, tk, woff:],
                            in_=E[:, tk, woff:],
                            pattern=[[1, 512 - woff]],
                            compare_op=mybir.AluOpType.is_ge,
                            fill=0.0,
                            base=woff - off,
                            channel_multiplier=-1,
                        )
                    if tk == 0:
                        nc.vector.tensor_copy(accE[:], E[:, 0, :])
                    else:
                        nc.vector.tensor_add(
                            accE[:, woff:], accE[:, woff:], E[:, tk, woff:]
                        )

            def pv_pair(h, qc, tkp, E, po, nk):
                woff = dg_off(qc, tkp * 2)
                for u in range(2):
                    tk = tkp * 2 + u
                    nc.tensor.matmul(
                        po[:, woff:], v_sb[:, tk, :], E[:, tk, woff:],
                        start=(tk == 0), stop=(tk == nk - 1),
                    )

            def sums_b(h, qc, accE, po, nT):
                sumsB = psfP.tile([128, 512], F32, tag="pf", name="sumsBt")
                nc.tensor.matmul(
                    sumsB[:], allones[:], accE[:], start=True, stop=True
                )
                recipS = rcp.tile([128, 512], F32, tag="recip", name="recipSt")
                nc.vector.reciprocal(recipS[:], sumsB[:])
                nc.vector.tensor_mul(nT[:, h, :], po[:], recipS[:])

            def o_tile(qc_prev, idx, drain_eng):
                """One O-projection output tile: 4 matmuls + copy + DMA."""
                qt, ecol = divmod(idx, 4)
                nT = nT_for[qc_prev]
                pf = psfP.tile([128, 512], F32, tag="pf", name="pft")
                for h in range(HPG):
                    nc.tensor.matmul(
                        pf[:],
                        nT[:, h, qt * 128:(qt + 1) * 128],
                        wo_sb[:, h, ecol * 512:(ecol + 1) * 512],
                        start=(h == 0),
                        stop=(h == HPG - 1),
                    )
                f_t = fsb.tile([128, 512], BF, tag="f", name="ftt")
                if drain_eng == 0:
                    nc.scalar.activation(f_t[:], pf[:], AF.Identity)
                else:
                    nc.vector.tensor_copy(f_t[:], pf[:])
                row = qc_prev * 4 + qt
                nc.sync.dma_start(
                    out_d[row * 128:(row + 1) * 128,
                          ecol * 512:(ecol + 1) * 512],
                    f_t[:],
                )

            # Global depth-2 pipeline over tasks (qc, h): S(task i) pairs
            # interleave with PV(task i-2) pairs, with O(qc-1) filler tiles
            # inserted by a deficit model (ACT exp needs ~1140ns per pair vs
            # ~426ns of PE work per S or PV pair; an O tile is ~852ns).
            tasks = [(qc, h) for qc in range(TC) for h in range(HPG)]
            NT = len(tasks)
            npair_of = lambda i: 2 * (tasks[i][0] + 1)
            nT_for[0] = nT_a
            nT_for[1] = nT_b
            nT_for[2] = nT_a
            nT_for[3] = nT_b

            Es, accEs, pos = {}, {}, {}

            def get_e(i):
                if i not in Es:
                    Es[i] = ebp.tile([128, 16, 512], BF, tag="E", name=f"E{i}")
                    accEs[i] = acp.tile([128, 512], BF, tag="acc",
                                        name=f"acc{i}")
                return Es[i], accEs[i]

            def get_po(i):
                if i not in pos:
                    pos[i] = psoP.tile([128, 512], F32, tag="po", name=f"po{i}")
                return pos[i]

            state = {"deficit": 0.0, "drain": 0, "fillers": [], "fi": 0,
                     "fqc": None}

            def fill_one():
                if state["fi"] < len(state["fillers"]):
                    o_tile(state["fqc"], state["fillers"][state["fi"]],
                           state["drain"])
                    state["drain"] ^= 1
                    state["fi"] += 1
                    state["deficit"] -= 852.0
                    return True
                return False

            def drain_deficit(slack):
                while state["deficit"] > slack and fill_one():
                    pass

            def emit_s(i, p):
                qc, h = tasks[i]
                E, accE = get_e(i)
                s_pair(h, qc, p, 4 * (qc + 1), E, accE)
                state["deficit"] += 1140.0 - 426.0

            def emit_pv(i, p):
                qc, h = tasks[i]
                pv_pair(h, qc, p, Es[i], get_po(i), 4 * (qc + 1))
                state["deficit"] -= 426.0

            def emit_sums(i):
                qc, h = tasks[i]
                sums_b(h, qc, accEs[i], pos[i], nT_for[qc])

            # interleave q half-passes with qc0 attention tasks so the
            # exp latency of the smallest q-chunk hides under projections
            q_half(0, 0)
            q_half(0, 1)
            q_half(1, 0)
            q_half(1, 1)
            for p in range(npair_of(0)):
                emit_s(0, p)
            q_half(2, 0)
            for p in range(npair_of(1)):
                emit_s(1, p)
                emit_pv(0, p)
            emit_sums(0)
            q_half(2, 1)
            q_half(3, 0)
            for p in range(npair_of(2)):
                emit_s(2, p)
                emit_pv(1, p)
            emit_sums(1)
            q_half(3, 1)
            state["deficit"] = 0.0

            for i in range(3, NT + 1):
                if i < NT:
                    qc, h = tasks[i]
                    if h == 1 and qc > 0:
                        # O(qc-1) fillers become legal here: sums(qc-1, h3)
                        # was emitted at the end of the previous task, so the
                        # whole nT(qc-1) buffer has its writers queued.  Old
                        # leftovers must flush first (their nT buffer gets
                        # overwritten by sums(qc, h0) at the end of this
                        # task).
                        while fill_one():
                            pass
                        state["fillers"] = list(range(16))
                        state["fi"] = 0
                        state["fqc"] = qc - 1
                    for p in range(npair_of(i)):
                        emit_s(i, p)
                        if i >= 1 and p < npair_of(i - 1):
                            emit_pv(i - 1, p)
                        drain_deficit(426.0)
                else:
                    for p in range(npair_of(i - 1)):
                        emit_pv(i - 1, p)
                        drain_deficit(0.0)
                if i >= 1:
                    emit_sums(i - 1)

            # tail: O-projection for the last q-chunk
            while fill_one():
                pass
            state["fillers"] = list(range(16))
            state["fi"] = 0
            state["fqc"] = TC - 1
            while fill_one():
                pass

            psfP.release()
            psoP.release()
            ring1.release()

    nc.compile()
    _PROG["nc"] = nc
    return nc


def prepare_in_maps(x, Wq, bq, Wk, bk, Wv, bv, Wo, bo):
    bf = ml_dtypes.bfloat16

    def to_sb_layout(W):  # [E, cols] -> [128, ECH*cols] partition-major
        cols = W.shape[1]
        return np.ascontiguousarray(
            W.reshape(ECH, 128, cols).transpose(1, 0, 2).reshape(128, ECH * cols)
        ).astype(bf)

    in_maps = []
    for c in range(NCORES):
        b, g = c // 4, c % 4
        xt = np.ascontiguousarray(np.asarray(x[b]).T).astype(bf).reshape(
            ECH, 128, T
        )
        wq = np.stack(
            [
                to_sb_layout(np.asarray(Wq[:, g * 512 + j * D: g * 512 + (j + 1) * D]))
                for j in range(HPG)
            ],
            axis=0,
        ).reshape(HPG, 128, ECH * D)
        wk = to_sb_layout(np.asarray(Wk[:, g * D:(g + 1) * D]))
        wv = to_sb_layout(np.asarray(Wv[:, g * D:(g + 1) * D]))
        wo = np.ascontiguousarray(Wo[g * 512:(g + 1) * 512, :]).astype(bf).reshape(
            HPG, 128, EMBED
        )
        bqc = np.ascontiguousarray(
            bq[g * 512:(g + 1) * 512].reshape(HPG, 128).T
        ).astype(np.float32)
        bkc = np.asarray(bk[g * D:(g + 1) * D]).reshape(128, 1).astype(np.float32)
        bvc = np.asarray(bv[g * D:(g + 1) * D]).reshape(1, 128).astype(bf)
        in_maps.append(
            {
                "xt": xt,
                "wq": wq,
                "wk": wk,
                "wv": wv,
                "wo": wo,
                "bq": bqc,
                "bk": bkc,
                "bv": bvc,
            }
        )
    return in_maps


def combine_outputs(results, bo):
    out = np.empty((2, T, EMBED), dtype=np.float32)
    for b in range(2):
        acc = results[b * 4]["out"].astype(np.float32)
        for g in range(1, 4):
            acc += results[b * 4 + g]["out"].astype(np.float32)
        out[b] = acc + np.asarray(bo)[None, :].astype(np.float32)
    return out


def kernel(x, Wq, bq, Wk, bk, Wv, bv, Wo, bo):
    from concourse.bass_utils import run_bass_kernel_spmd

    nc = build_program()
    in_maps = prepare_in_maps(x, Wq, bq, Wk, bk, Wv, bv, Wo, bo)
    res = run_bass_kernel_spmd(nc, in_maps, list(range(NCORES)))
    return combine_outputs(res.results, np.asarray(bo))


# revision 3
# speedup vs baseline: 1.0025x; 1.0025x over previous
"""Grouped-Query Attention on 8 Trainium2 NeuronCores — v2.

Sharding: TP-4 over KV groups x DP-2 over batch.
Core c handles batch b = c // 4, group g = c % 4 (4 query heads, 1 KV group).

Differences vs v1:
  - V is projected directly into natural [t, d] layout in phase 1
    (stationary = xt tile, moving = Wv chunk) -- no PE transposes.
  - Softmax denominators: E accumulated over tk on DVE (bf16 2x), then a
    single allones[128,128] @ accE matmul broadcasts the denominator --
    removes 160 sum-matmuls + 16 broadcast-matmuls from PE.
  - Causal masking via Pool-engine affine_select (in-place on E).
  - Software-pipelined PE emission: S(h) pairs interleave with PV(h-1)
    pairs and O-projection(qc-1) filler matmuls.
  - DMA: wq split per head and interleaved with the xt stream; weights
    pre-transposed on host for 4KB-contiguous descriptors.
"""

import numpy as np
import ml_dtypes

EMBED = 2048
T = 2048
D = 128           # head dim
NQH = 16          # query heads
NG = 4            # kv groups
HPG = NQH // NG   # query heads per group = 4
NCORES = 8
ECH = EMBED // 128   # 16 contraction chunks
TC = T // 512        # 4 t-chunks of 512
TT = T // 128        # 16 t-tiles of 128
SCALE = 1.0 / float(np.sqrt(D))

_PROG = {}


def build_program():
    if "nc" in _PROG:
        return _PROG["nc"]

    from contextlib import ExitStack
    import concourse.mybir as mybir
    from concourse import bacc, tile

    # Drop redundant consecutive Ldweights with identical keys (the Tile
    # legalizer emits one per Matmult even when the stationary is unchanged).
    if not getattr(tile.tile_legalize, "_ldw_dedup", False):
        _orig_legalize = tile.tile_legalize

        def _dedup_legalize(ordered, nc_):
            ordered = _orig_legalize(ordered, nc_)
            for bb, insts in ordered.items():
                out = []
                state = None
                for inst in insts:
                    tn = type(inst).__name__
                    if tn == "InstLdweights":
                        key = (
                            str(inst.ins[0]),
                            str(getattr(inst, "is_transpose", None)),
                            str(getattr(inst, "tile_position", None)),
                            str(getattr(inst, "perf_mode", None)),
                        )
                        if key == state:
                            continue
                        state = key
                    out.append(inst)
                ordered[bb] = out
            return ordered

        _dedup_legalize._ldw_dedup = True
        tile.tile_legalize = _dedup_legalize

    dt = mybir.dt
    BF = dt.bfloat16
    F32 = dt.float32
    AF = mybir.ActivationFunctionType

    nc = bacc.Bacc("TRN2", target_bir_lowering=False, debug=False)

    xt_d = nc.dram_tensor("xt", [ECH, 128, T], BF, kind="ExternalInput")
    wq_d = nc.dram_tensor("wq", [HPG, 128, ECH * D], BF, kind="ExternalInput")
    wk_d = nc.dram_tensor("wk", [128, ECH * D], BF, kind="ExternalInput")
    wv_d = nc.dram_tensor("wv", [128, ECH * D], BF, kind="ExternalInput")
    wo_d = nc.dram_tensor("wo", [HPG, 128, EMBED], BF, kind="ExternalInput")
    bq_d = nc.dram_tensor("bq", [128, HPG], F32, kind="ExternalInput")
    bk_d = nc.dram_tensor("bk", [128, 1], F32, kind="ExternalInput")
    bv_d = nc.dram_tensor("bv", [1, 128], BF, kind="ExternalInput")
    out_d = nc.dram_tensor("out", [T, EMBED], BF, kind="ExternalOutput")

    with tile.TileContext(nc) as tc, ExitStack() as ctx:
        pers = ctx.enter_context(tc.tile_pool(name="pers", bufs=1))

        wq_sb = pers.tile([128, HPG, ECH * D], BF)
        wk_sb = pers.tile([128, ECH * D], BF)
        wv_sb = pers.tile([128, ECH * D], BF)
        wo_sb = pers.tile([128, HPG, EMBED], BF)
        bq_sb = pers.tile([128, HPG], F32)
        bk_sb = pers.tile([128, 1], F32)
        bv_sb = pers.tile([1, 128], BF)
        qT_sb = pers.tile([128, HPG, T], BF)
        kT_sb = pers.tile([128, T], BF)
        v_sb = pers.tile([128, TT, D], BF)
        nT_a = pers.tile([128, HPG, 512], BF)
        nT_b = pers.tile([128, HPG, 512], BF)
        ones1 = pers.tile([1, 128], BF)
        allones = pers.tile([128, 128], BF)

        nc.gpsimd.memset(ones1[:], 1.0)
        nc.gpsimd.memset(allones[:], 1.0)

        # ---- Phase 1: projections ----
        if True:
            xt_sb = pers.tile([128, ECH, T], BF)

            # All input DMAs on the sync queue in priority order (the DMA
            # engines are a shared serial resource): wk/wv + biases first,
            # then xt chunks with wq heads interleaved, wo last (needed only
            # by the O-projection ~60us later).
            nc.sync.dma_start(wk_sb[:, 0:D], wk_d[:, 0:D])
            nc.sync.dma_start(xt_sb[:, 0, 0:1024], xt_d[0][:, 0:1024])
            nc.sync.dma_start(wv_sb[:, 0:D], wv_d[:, 0:D])
            nc.sync.dma_start(xt_sb[:, 0, 1024:], xt_d[0][:, 1024:])
            nc.sync.dma_start(wk_sb[:, D:], wk_d[:, D:])
            nc.sync.dma_start(xt_sb[:, 1, :], xt_d[1])
            nc.sync.dma_start(wv_sb[:, D:], wv_d[:, D:])
            nc.sync.dma_start(xt_sb[:, 2, :], xt_d[2])
            nc.sync.dma_start(bk_sb[:], bk_d[:])
            nc.sync.dma_start(bv_sb[:], bv_d[:])
            nc.sync.dma_start(bq_sb[:], bq_d[:])
            wq_next = 0
            for ec in range(3, ECH):
                nc.sync.dma_start(xt_sb[:, ec, :], xt_d[ec])
                if ec % 6 == 2 and wq_next < HPG:
                    nc.sync.dma_start(wq_sb[:, wq_next, :], wq_d[wq_next])
                    wq_next += 1
            while wq_next < HPG:
                nc.sync.dma_start(wq_sb[:, wq_next, :], wq_d[wq_next])
                wq_next += 1
            nc.sync.dma_start(wo_sb[:], wo_d.ap().rearrange("h p e -> p h e"))

            # --- phase-1 PSUM: v accumulates on the right stack, kT on
            # the left.  kT's pool releases into ring1 (q half-passes + s2
            # pairs); v's pool releases into the po/pf pools.  No pool
            # barrier separates phase 1 from attention.
            kvB = tc.alloc_tile_pool(name="kvB", bufs=1, space="PSUM",
                                     side="right")
            vps = kvB.tile([128, TT, D], F32, tag="v", name="vps")
            kvA = tc.alloc_tile_pool(name="kvA", bufs=1, space="PSUM",
                                     side="left")
            kps = kvA.tile([128, T], F32, tag="k", name="kps")
            for ec in range(ECH):
                wkc = wk_sb[:, ec * D:(ec + 1) * D]
                for t5 in range(TC):
                    nc.tensor.matmul(
                        kps[:, t5 * 512:(t5 + 1) * 512],
                        wkc,
                        xt_sb[:, ec, t5 * 512:(t5 + 1) * 512],
                        start=(ec == 0),
                        stop=(ec == ECH - 1),
                    )
                wvc = wv_sb[:, ec * D:(ec + 1) * D]
                for tt in range(TT):
                    # 4 tt-tiles share a PSUM bank; `start` zeroes the whole
                    # 2KB bank region, so only the bank's first write starts
                    # the group and its last write stops it.
                    nc.tensor.matmul(
                        vps[:, tt, :],
                        xt_sb[:, ec, tt * D:(tt + 1) * D],
                        wvc,
                        start=(ec == 0 and tt % 4 == 0),
                        stop=False,
                    )
            # fold bv in as a rank-1 accumulation, closing each group
            for tt in range(TT):
                nc.tensor.matmul(
                    vps[:, tt, :], ones1[:], bv_sb[:],
                    start=False, stop=(tt % 4 == 3),
                )
            # drains: kT spread over ACT/DVE/Pool; v split DVE/Pool
            for t5 in range(TC):
                sl = slice(t5 * 512, (t5 + 1) * 512)
                nc.scalar.activation(
                    kT_sb[:, sl], kps[:, sl], AF.Identity, bias=bk_sb[:]
                )
            for tt in range(TT):
                nc.vector.tensor_copy(v_sb[:, tt, :], vps[:, tt, :])
            kvA.release()
            ring1 = tc.alloc_tile_pool(name="ring1", bufs=2, space="PSUM",
                                       side="left")
            kvB.release()
            psoP = tc.alloc_tile_pool(name="psoP", bufs=2, space="PSUM",
                                      side="right")
            psfP = tc.alloc_tile_pool(name="psfP", bufs=2, space="PSUM",
                                      side="right")

            def q_half(j, th):
                ps = ring1.tile([128, 1024], F32, tag="r", name=f"q{j}h{th}")
                for ec in range(ECH):
                    lhsT = wq_sb[:, j, ec * D:(ec + 1) * D]
                    for t5 in range(2):
                        lo = th * 1024 + t5 * 512
                        nc.tensor.matmul(
                            ps[:, t5 * 512:(t5 + 1) * 512],
                            lhsT,
                            xt_sb[:, ec, lo:lo + 512],
                            start=(ec == 0),
                            stop=(ec == ECH - 1),
                        )
                if th == 0:
                    nc.scalar.activation(
                        qT_sb[:, j, th * 1024:(th + 1) * 1024], ps[:],
                        AF.Identity, bias=bq_sb[:, j:j + 1],
                    )
                else:
                    nc.vector.tensor_scalar_add(
                        qT_sb[:, j, th * 1024:(th + 1) * 1024], ps[:],
                        bq_sb[:, j:j + 1],
                    )

        # ---- Phase 2: attention + O-projection, software-pipelined ----
        with (
            tc.tile_pool(name="eb", bufs=2) as ebp,
            tc.tile_pool(name="acp", bufs=2) as acp,
            tc.tile_pool(name="rcp", bufs=2) as rcp,
            tc.tile_pool(name="fsb", bufs=4) as fsb,
        ):
            nT_for = {}

            def dg_off(qc, tk):
                # columns [0, off) of tile tk are fully below the causal
                # diagonal (masked out) -- skip computing them entirely
                return max(0, (tk - 4 * qc)) * D if tk >= 4 * qc else 0

            def s_pair(h, qc, tkp, nk, E, accE):
                """Two S matmuls -> exp pair -> mask diag -> accumulate."""
                s2 = ring1.tile([128, 2, 512], F32, tag="r", name="s2t")
                # both tiles of a pair computed at the pair's min diagonal
                # offset so the exp reads a fully-written region
                woff = dg_off(qc, tkp * 2)
                for u in range(2):
                    tk = tkp * 2 + u
                    nc.tensor.matmul(
                        s2[:, u, woff:],
                        kT_sb[:, tk * D:(tk + 1) * D],
                        qT_sb[:, h, qc * 512 + woff:(qc + 1) * 512],
                        start=True,
                        stop=True,
                    )
                nc.scalar.activation(
                    E[:, tkp * 2:tkp * 2 + 2, woff:], s2[:, :, woff:],
                    AF.Exp, scale=SCALE
                )
                for u in range(2):
                    tk = tkp * 2 + u
                    off = dg_off(qc, tk)
                    if tk >= 4 * qc:
                        # zero E[p, woff+f] where woff + f < off + p
                        nc.gpsimd.affine_select(
                            out=E[:, tk, woff:],
                            in_=E[:, tk, woff:],
                            pattern=[[1, 512 - woff]],
                            compare_op=mybir.AluOpType.is_ge,
                            fill=0.0,
                            base=woff - off,
                            channel_multiplier=-1,
                        )
                    if tk == 0:
                        nc.vector.tensor_copy(accE[:], E[:, 0, :])
                    else:
                        nc.vector.tensor_add(
                            accE[:, woff:], accE[:, woff:], E[:, tk, woff:]
                        )

            def pv_pair(h, qc, tkp, E, po, nk):
                woff = dg_off(qc, tkp * 2)
                for u in range(2):
                    tk = tkp * 2 + u
                    nc.tensor.matmul(
                        po[:, woff:], v_sb[:, tk, :], E[:, tk, woff:],
                        start=(tk == 0), stop=(tk == nk - 1),
                    )

            def sums_b(h, qc, accE, po, nT):
                sumsB = psfP.tile([128, 512], F32, tag="pf", name="sumsBt")
                nc.tensor.matmul(
                    sumsB[:], allones[:], accE[:], start=True, stop=True
                )
                recipS = rcp.tile([128, 512], F32, tag="recip", name="recipSt")
                nc.vector.reciprocal(recipS[:], sumsB[:])
                nc.vector.tensor_mul(nT[:, h, :], po[:], recipS[:])

            def o_tile(qc_prev, idx, drain_eng, split_drain=False):
                """One O-projection output tile: 4 matmuls + copy + DMA."""
                qt, ecol = divmod(idx, 4)
                nT = nT_for[qc_prev]
                pf = psfP.tile([128, 512], F32, tag="pf", name="pft")
                for h in range(HPG):
                    nc.tensor.matmul(
                        pf[:],
                        nT[:, h, qt * 128:(qt + 1) * 128],
                        wo_sb[:, h, ecol * 512:(ecol + 1) * 512],
                        start=(h == 0),
                        stop=(h == HPG - 1),
                    )
                f_t = fsb.tile([128, 512], BF, tag="f", name="ftt")
                row = qc_prev * 4 + qt
                if split_drain:
                    # end-of-program: halve latency by draining on both
                    # engines and overlapping the two half DMAs
                    nc.scalar.activation(f_t[:, :256], pf[:, :256], AF.Identity)
                    nc.vector.tensor_copy(f_t[:, 256:], pf[:, 256:])
                    nc.sync.dma_start(
                        out_d[row * 128:(row + 1) * 128,
                              ecol * 512:ecol * 512 + 256],
                        f_t[:, :256],
                    )
                    nc.sync.dma_start(
                        out_d[row * 128:(row + 1) * 128,
                              ecol * 512 + 256:(ecol + 1) * 512],
                        f_t[:, 256:],
                    )
                    return
                if drain_eng == 0:
                    nc.scalar.activation(f_t[:], pf[:], AF.Identity)
                else:
                    nc.vector.tensor_copy(f_t[:], pf[:])
                nc.sync.dma_start(
                    out_d[row * 128:(row + 1) * 128,
                          ecol * 512:(ecol + 1) * 512],
                    f_t[:],
                )

            # Global depth-2 pipeline over tasks (qc, h): S(task i) pairs
            # interleave with PV(task i-2) pairs, with O(qc-1) filler tiles
            # inserted by a deficit model (ACT exp needs ~1140ns per pair vs
            # ~426ns of PE work per S or PV pair; an O tile is ~852ns).
            tasks = [(qc, h) for qc in range(TC) for h in range(HPG)]
            NT = len(tasks)
            npair_of = lambda i: 2 * (tasks[i][0] + 1)
            nT_for[0] = nT_a
            nT_for[1] = nT_b
            nT_for[2] = nT_a
            nT_for[3] = nT_b

            Es, accEs, pos = {}, {}, {}

            def get_e(i):
                if i not in Es:
                    Es[i] = ebp.tile([128, 16, 512], BF, tag="E", name=f"E{i}")
                    accEs[i] = acp.tile([128, 512], BF, tag="acc",
                                        name=f"acc{i}")
                return Es[i], accEs[i]

            def get_po(i):
                if i not in pos:
                    pos[i] = psoP.tile([128, 512], F32, tag="po", name=f"po{i}")
                return pos[i]

            state = {"deficit": 0.0, "drain": 0, "fillers": [], "fi": 0,
                     "fqc": None}

            def fill_one():
                if state["fi"] < len(state["fillers"]):
                    o_tile(state["fqc"], state["fillers"][state["fi"]],
                           state["drain"])
                    state["drain"] ^= 1
                    state["fi"] += 1
                    state["deficit"] -= 852.0
                    return True
                return False

            def drain_deficit(slack):
                while state["deficit"] > slack and fill_one():
                    pass

            def emit_s(i, p):
                qc, h = tasks[i]
                E, accE = get_e(i)
                s_pair(h, qc, p, 4 * (qc + 1), E, accE)
                state["deficit"] += 1140.0 - 426.0

            def emit_pv(i, p):
                qc, h = tasks[i]
                pv_pair(h, qc, p, Es[i], get_po(i), 4 * (qc + 1))
                state["deficit"] -= 426.0

            def emit_sums(i):
                qc, h = tasks[i]
                sums_b(h, qc, accEs[i], pos[i], nT_for[qc])

            # interleave q half-passes with qc0 attention tasks so the
            # exp latency of the smallest q-chunk hides under projections
            q_half(0, 0)
            q_half(0, 1)
            q_half(1, 0)
            q_half(1, 1)
            for p in range(npair_of(0)):
                emit_s(0, p)
            q_half(2, 0)
            for p in range(npair_of(1)):
                emit_s(1, p)
                emit_pv(0, p)
            emit_sums(0)
            q_half(2, 1)
            q_half(3, 0)
            for p in range(npair_of(2)):
                emit_s(2, p)
                emit_pv(1, p)
            emit_sums(1)
            q_half(3, 1)
            state["deficit"] = 0.0

            for i in range(3, NT + 1):
                if i < NT:
                    qc, h = tasks[i]
                    if h == 1 and qc > 0:
                        # O(qc-1) fillers become legal here: sums(qc-1, h3)
                        # was emitted at the end of the previous task, so the
                        # whole nT(qc-1) buffer has its writers queued.  Old
                        # leftovers must flush first (their nT buffer gets
                        # overwritten by sums(qc, h0) at the end of this
                        # task).
                        while fill_one():
                            pass
                        state["fillers"] = list(range(16))
                        state["fi"] = 0
                        state["fqc"] = qc - 1
                    for p in range(npair_of(i)):
                        emit_s(i, p)
                        if i >= 1 and p < npair_of(i - 1):
                            emit_pv(i - 1, p)
                        drain_deficit(426.0)
                else:
                    for p in range(npair_of(i - 1)):
                        emit_pv(i - 1, p)
                        drain_deficit(0.0)
                if i >= 1:
                    emit_sums(i - 1)

            # tail: O-projection for the last q-chunk
            while fill_one():
                pass
            for idx in range(14):
                o_tile(TC - 1, idx, idx & 1)
            for idx in (14, 15):
                o_tile(TC - 1, idx, 0, split_drain=True)

            psfP.release()
            psoP.release()
            ring1.release()

    nc.compile()
    _PROG["nc"] = nc
    return nc


def prepare_in_maps(x, Wq, bq, Wk, bk, Wv, bv, Wo, bo):
    bf = ml_dtypes.bfloat16

    def to_sb_layout(W):  # [E, cols] -> [128, ECH*cols] partition-major
        cols = W.shape[1]
        return np.ascontiguousarray(
            W.reshape(ECH, 128, cols).transpose(1, 0, 2).reshape(128, ECH * cols)
        ).astype(bf)

    in_maps = []
    for c in range(NCORES):
        b, g = c // 4, c % 4
        xt = np.ascontiguousarray(np.asarray(x[b]).T).astype(bf).reshape(
            ECH, 128, T
        )
        wq = np.stack(
            [
                to_sb_layout(np.asarray(Wq[:, g * 512 + j * D: g * 512 + (j + 1) * D]))
                for j in range(HPG)
            ],
            axis=0,
        ).reshape(HPG, 128, ECH * D)
        wk = to_sb_layout(np.asarray(Wk[:, g * D:(g + 1) * D]))
        wv = to_sb_layout(np.asarray(Wv[:, g * D:(g + 1) * D]))
        wo = np.ascontiguousarray(Wo[g * 512:(g + 1) * 512, :]).astype(bf).reshape(
            HPG, 128, EMBED
        )
        bqc = np.ascontiguousarray(
            bq[g * 512:(g + 1) * 512].reshape(HPG, 128).T
        ).astype(np.float32)
        bkc = np.asarray(bk[g * D:(g + 1) * D]).reshape(128, 1).astype(np.float32)
        bvc = np.asarray(bv[g * D:(g + 1) * D]).reshape(1, 128).astype(bf)
        in_maps.append(
            {
                "xt": xt,
                "wq": wq,
                "wk": wk,
                "wv": wv,
                "wo": wo,
                "bq": bqc,
                "bk": bkc,
                "bv": bvc,
            }
        )
    return in_maps


def combine_outputs(results, bo):
    out = np.empty((2, T, EMBED), dtype=np.float32)
    for b in range(2):
        acc = results[b * 4]["out"].astype(np.float32)
        for g in range(1, 4):
            acc += results[b * 4 + g]["out"].astype(np.float32)
        out[b] = acc + np.asarray(bo)[None, :].astype(np.float32)
    return out


def kernel(x, Wq, bq, Wk, bk, Wv, bv, Wo, bo):
    from concourse.bass_utils import run_bass_kernel_spmd

    nc = build_program()
    in_maps = prepare_in_maps(x, Wq, bq, Wk, bk, Wv, bv, Wo, bo)
    res = run_bass_kernel_spmd(nc, in_maps, list(range(NCORES)))
    return combine_outputs(res.results, np.asarray(bo))


# revision 4
# speedup vs baseline: 1.0028x; 1.0003x over previous
"""Grouped-Query Attention on 8 Trainium2 NeuronCores — v2.

Sharding: TP-4 over KV groups x DP-2 over batch.
Core c handles batch b = c // 4, group g = c % 4 (4 query heads, 1 KV group).

Differences vs v1:
  - V is projected directly into natural [t, d] layout in phase 1
    (stationary = xt tile, moving = Wv chunk) -- no PE transposes.
  - Softmax denominators: E accumulated over tk on DVE (bf16 2x), then a
    single allones[128,128] @ accE matmul broadcasts the denominator --
    removes 160 sum-matmuls + 16 broadcast-matmuls from PE.
  - Causal masking via Pool-engine affine_select (in-place on E).
  - Software-pipelined PE emission: S(h) pairs interleave with PV(h-1)
    pairs and O-projection(qc-1) filler matmuls.
  - DMA: wq split per head and interleaved with the xt stream; weights
    pre-transposed on host for 4KB-contiguous descriptors.
"""

import numpy as np
import ml_dtypes

EMBED = 2048
T = 2048
D = 128           # head dim
NQH = 16          # query heads
NG = 4            # kv groups
HPG = NQH // NG   # query heads per group = 4
NCORES = 8
ECH = EMBED // 128   # 16 contraction chunks
TC = T // 512        # 4 t-chunks of 512
TT = T // 128        # 16 t-tiles of 128
SCALE = 1.0 / float(np.sqrt(D))

_PROG = {}


def build_program():
    if "nc" in _PROG:
        return _PROG["nc"]

    from contextlib import ExitStack
    import concourse.mybir as mybir
    from concourse import bacc, tile

    # Drop redundant consecutive Ldweights with identical keys (the Tile
    # legalizer emits one per Matmult even when the stationary is unchanged).
    if not getattr(tile.tile_legalize, "_ldw_dedup", False):
        _orig_legalize = tile.tile_legalize

        def _dedup_legalize(ordered, nc_):
            ordered = _orig_legalize(ordered, nc_)
            for bb, insts in ordered.items():
                out = []
                state = None
                for inst in insts:
                    tn = type(inst).__name__
                    if tn == "InstLdweights":
                        key = (
                            str(inst.ins[0]),
                            str(getattr(inst, "is_transpose", None)),
                            str(getattr(inst, "tile_position", None)),
                            str(getattr(inst, "perf_mode", None)),
                        )
                        if key == state:
                            continue
                        state = key
                    out.append(inst)
                ordered[bb] = out
            return ordered

        _dedup_legalize._ldw_dedup = True
        tile.tile_legalize = _dedup_legalize

    dt = mybir.dt
    BF = dt.bfloat16
    F32 = dt.float32
    AF = mybir.ActivationFunctionType

    nc = bacc.Bacc("TRN2", target_bir_lowering=False, debug=False)

    xt_d = nc.dram_tensor("xt", [ECH, 128, T], BF, kind="ExternalInput")
    wq_d = nc.dram_tensor("wq", [HPG, 128, ECH * D], BF, kind="ExternalInput")
    wk_d = nc.dram_tensor("wk", [128, ECH * D], BF, kind="ExternalInput")
    wv_d = nc.dram_tensor("wv", [128, ECH * D], BF, kind="ExternalInput")
    wo_d = nc.dram_tensor("wo", [HPG, 128, EMBED], BF, kind="ExternalInput")
    bq_d = nc.dram_tensor("bq", [128, HPG], F32, kind="ExternalInput")
    bk_d = nc.dram_tensor("bk", [128, 1], F32, kind="ExternalInput")
    bv_d = nc.dram_tensor("bv", [1, 128], BF, kind="ExternalInput")
    out_d = nc.dram_tensor("out", [T, EMBED], BF, kind="ExternalOutput")

    with tile.TileContext(nc) as tc, ExitStack() as ctx:
        pers = ctx.enter_context(tc.tile_pool(name="pers", bufs=1))

        wq_sb = pers.tile([128, HPG, ECH * D], BF)
        wk_sb = pers.tile([128, ECH * D], BF)
        wv_sb = pers.tile([128, ECH * D], BF)
        wo_sb = pers.tile([128, HPG, EMBED], BF)
        bq_sb = pers.tile([128, HPG], F32)
        bk_sb = pers.tile([128, 1], F32)
        bv_sb = pers.tile([1, 128], BF)
        qT_sb = pers.tile([128, HPG, T], BF)
        kT_sb = pers.tile([128, T], BF)
        v_sb = pers.tile([128, TT, D], BF)
        nT_a = pers.tile([128, HPG, 512], BF)
        nT_b = pers.tile([128, HPG, 512], BF)
        ones1 = pers.tile([1, 128], BF)
        allones = pers.tile([128, 128], BF)

        nc.gpsimd.memset(ones1[:], 1.0)
        nc.gpsimd.memset(allones[:], 1.0)

        # ---- Phase 1: projections ----
        if True:
            xt_sb = pers.tile([128, ECH, T], BF)

            # All input DMAs on the sync queue in priority order (the DMA
            # engines are a shared serial resource): wk/wv + biases first,
            # then xt chunks with wq heads interleaved, wo last (needed only
            # by the O-projection ~60us later).
            nc.sync.dma_start(wk_sb[:, 0:D], wk_d[:, 0:D])
            nc.sync.dma_start(xt_sb[:, 0, 0:1024], xt_d[0][:, 0:1024])
            nc.sync.dma_start(wv_sb[:, 0:D], wv_d[:, 0:D])
            nc.sync.dma_start(xt_sb[:, 0, 1024:], xt_d[0][:, 1024:])
            nc.sync.dma_start(wk_sb[:, D:], wk_d[:, D:])
            nc.sync.dma_start(xt_sb[:, 1, :], xt_d[1])
            nc.sync.dma_start(wv_sb[:, D:], wv_d[:, D:])
            nc.sync.dma_start(xt_sb[:, 2, :], xt_d[2])
            nc.sync.dma_start(bk_sb[:], bk_d[:])
            nc.sync.dma_start(bv_sb[:], bv_d[:])
            nc.sync.dma_start(bq_sb[:], bq_d[:])
            wq_next = 0
            for ec in range(3, ECH):
                nc.sync.dma_start(xt_sb[:, ec, :], xt_d[ec])
                if ec % 6 == 2 and wq_next < HPG:
                    nc.sync.dma_start(wq_sb[:, wq_next, :], wq_d[wq_next])
                    wq_next += 1
            while wq_next < HPG:
                nc.sync.dma_start(wq_sb[:, wq_next, :], wq_d[wq_next])
                wq_next += 1
            nc.sync.dma_start(wo_sb[:], wo_d.ap().rearrange("h p e -> p h e"))

            # --- phase-1 PSUM: v accumulates on the right stack, kT on
            # the left.  kT's pool releases into ring1 (q half-passes + s2
            # pairs); v's pool releases into the po/pf pools.  No pool
            # barrier separates phase 1 from attention.
            kvB = tc.alloc_tile_pool(name="kvB", bufs=1, space="PSUM",
                                     side="right")
            vps = kvB.tile([128, TT, D], F32, tag="v", name="vps")
            kvA = tc.alloc_tile_pool(name="kvA", bufs=1, space="PSUM",
                                     side="left")
            kps = kvA.tile([128, T], F32, tag="k", name="kps")
            for ec in range(ECH):
                wkc = wk_sb[:, ec * D:(ec + 1) * D]
                for t5 in range(TC):
                    nc.tensor.matmul(
                        kps[:, t5 * 512:(t5 + 1) * 512],
                        wkc,
                        xt_sb[:, ec, t5 * 512:(t5 + 1) * 512],
                        start=(ec == 0),
                        stop=(ec == ECH - 1),
                    )
                wvc = wv_sb[:, ec * D:(ec + 1) * D]
                for tt in range(TT):
                    # 4 tt-tiles share a PSUM bank; `start` zeroes the whole
                    # 2KB bank region, so only the bank's first write starts
                    # the group and its last write stops it.
                    nc.tensor.matmul(
                        vps[:, tt, :],
                        xt_sb[:, ec, tt * D:(tt + 1) * D],
                        wvc,
                        start=(ec == 0 and tt % 4 == 0),
                        stop=False,
                    )
            # fold bv in as a rank-1 accumulation, closing each group
            for tt in range(TT):
                nc.tensor.matmul(
                    vps[:, tt, :], ones1[:], bv_sb[:],
                    start=False, stop=(tt % 4 == 3),
                )
            # drains: kT spread over ACT/DVE/Pool; v split DVE/Pool
            for t5 in range(TC):
                sl = slice(t5 * 512, (t5 + 1) * 512)
                nc.scalar.activation(
                    kT_sb[:, sl], kps[:, sl], AF.Identity, bias=bk_sb[:]
                )
            for tt in range(TT):
                nc.vector.tensor_copy(v_sb[:, tt, :], vps[:, tt, :])
            kvA.release()
            ring1 = tc.alloc_tile_pool(name="ring1", bufs=2, space="PSUM",
                                       side="left")
            kvB.release()
            psoP = tc.alloc_tile_pool(name="psoP", bufs=2, space="PSUM",
                                      side="right")
            psfP = tc.alloc_tile_pool(name="psfP", bufs=2, space="PSUM",
                                      side="right")

            def q_half(j, th):
                ps = ring1.tile([128, 1024], F32, tag="r", name=f"q{j}h{th}")
                for ec in range(ECH):
                    lhsT = wq_sb[:, j, ec * D:(ec + 1) * D]
                    for t5 in range(2):
                        lo = th * 1024 + t5 * 512
                        nc.tensor.matmul(
                            ps[:, t5 * 512:(t5 + 1) * 512],
                            lhsT,
                            xt_sb[:, ec, lo:lo + 512],
                            start=(ec == 0),
                            stop=(ec == ECH - 1),
                        )
                if th == 0:
                    nc.scalar.activation(
                        qT_sb[:, j, th * 1024:(th + 1) * 1024], ps[:],
                        AF.Identity, bias=bq_sb[:, j:j + 1],
                    )
                else:
                    nc.vector.tensor_scalar_add(
                        qT_sb[:, j, th * 1024:(th + 1) * 1024], ps[:],
                        bq_sb[:, j:j + 1],
                    )

        # ---- Phase 2: attention + O-projection, software-pipelined ----
        with (
            tc.tile_pool(name="eb", bufs=2) as ebp,
            tc.tile_pool(name="acp", bufs=2) as acp,
            tc.tile_pool(name="rcp", bufs=2) as rcp,
            tc.tile_pool(name="fsb", bufs=4) as fsb,
        ):
            nT_for = {}

            def dg_off(qc, tk):
                # columns [0, off) of tile tk are fully below the causal
                # diagonal (masked out) -- skip computing them entirely
                return max(0, (tk - 4 * qc)) * D if tk >= 4 * qc else 0

            def s_pair(h, qc, tkp, nk, E, accE):
                """Two S matmuls -> exp pair -> mask diag -> accumulate."""
                s2 = ring1.tile([128, 2, 512], F32, tag="r", name="s2t")
                # both tiles of a pair computed at the pair's min diagonal
                # offset so the exp reads a fully-written region
                woff = dg_off(qc, tkp * 2)
                for u in range(2):
                    tk = tkp * 2 + u
                    nc.tensor.matmul(
                        s2[:, u, woff:],
                        kT_sb[:, tk * D:(tk + 1) * D],
                        qT_sb[:, h, qc * 512 + woff:(qc + 1) * 512],
                        start=True,
                        stop=True,
                    )
                nc.scalar.activation(
                    E[:, tkp * 2:tkp * 2 + 2, woff:], s2[:, :, woff:],
                    AF.Exp, scale=SCALE
                )
                for u in range(2):
                    tk = tkp * 2 + u
                    off = dg_off(qc, tk)
                    if tk >= 4 * qc:
                        # zero E[p, woff+f] where woff + f < off + p
                        nc.gpsimd.affine_select(
                            out=E[:, tk, woff:],
                            in_=E[:, tk, woff:],
                            pattern=[[1, 512 - woff]],
                            compare_op=mybir.AluOpType.is_ge,
                            fill=0.0,
                            base=woff - off,
                            channel_multiplier=-1,
                        )
                    if tk == 0:
                        nc.vector.tensor_copy(accE[:], E[:, 0, :])
                    else:
                        nc.vector.tensor_add(
                            accE[:, woff:], accE[:, woff:], E[:, tk, woff:]
                        )

            def pv_pair(h, qc, tkp, E, po, nk):
                woff = dg_off(qc, tkp * 2)
                for u in range(2):
                    tk = tkp * 2 + u
                    nc.tensor.matmul(
                        po[:, woff:], v_sb[:, tk, :], E[:, tk, woff:],
                        start=(tk == 0), stop=(tk == nk - 1),
                    )

            def sums_b(h, qc, accE, po, nT):
                sumsB = psfP.tile([128, 512], F32, tag="pf", name="sumsBt")
                nc.tensor.matmul(
                    sumsB[:], allones[:], accE[:], start=True, stop=True
                )
                recipS = rcp.tile([128, 512], F32, tag="recip", name="recipSt")
                nc.vector.reciprocal(recipS[:], sumsB[:])
                nc.vector.tensor_mul(nT[:, h, :], po[:], recipS[:])

            def o_tile(qc_prev, idx, drain_eng, split_drain=False):
                """One O-projection output tile: 4 matmuls + copy + DMA."""
                qt, ecol = divmod(idx, 4)
                nT = nT_for[qc_prev]
                pf = psfP.tile([128, 512], F32, tag="pf", name="pft")
                for h in range(HPG):
                    nc.tensor.matmul(
                        pf[:],
                        nT[:, h, qt * 128:(qt + 1) * 128],
                        wo_sb[:, h, ecol * 512:(ecol + 1) * 512],
                        start=(h == 0),
                        stop=(h == HPG - 1),
                    )
                f_t = fsb.tile([128, 512], BF, tag="f", name="ftt")
                row = qc_prev * 4 + qt
                if split_drain:
                    # end-of-program: halve latency by draining on both
                    # engines and overlapping the two half DMAs
                    nc.scalar.activation(f_t[:, :256], pf[:, :256], AF.Identity)
                    nc.vector.tensor_copy(f_t[:, 256:], pf[:, 256:])
                    nc.sync.dma_start(
                        out_d[row * 128:(row + 1) * 128,
                              ecol * 512:ecol * 512 + 256],
                        f_t[:, :256],
                    )
                    nc.sync.dma_start(
                        out_d[row * 128:(row + 1) * 128,
                              ecol * 512 + 256:(ecol + 1) * 512],
                        f_t[:, 256:],
                    )
                    return
                if drain_eng == 0:
                    nc.scalar.activation(f_t[:], pf[:], AF.Identity)
                else:
                    nc.vector.tensor_copy(f_t[:], pf[:])
                nc.sync.dma_start(
                    out_d[row * 128:(row + 1) * 128,
                          ecol * 512:(ecol + 1) * 512],
                    f_t[:],
                )

            # Global depth-2 pipeline over tasks (qc, h): S(task i) pairs
            # interleave with PV(task i-2) pairs, with O(qc-1) filler tiles
            # inserted by a deficit model (ACT exp needs ~1140ns per pair vs
            # ~426ns of PE work per S or PV pair; an O tile is ~852ns).
            tasks = [(qc, h) for qc in range(TC) for h in range(HPG)]
            NT = len(tasks)
            npair_of = lambda i: 2 * (tasks[i][0] + 1)
            nT_for[0] = nT_a
            nT_for[1] = nT_b
            nT_for[2] = nT_a
            nT_for[3] = nT_b

            Es, accEs, pos = {}, {}, {}

            def get_e(i):
                if i not in Es:
                    Es[i] = ebp.tile([128, 16, 512], BF, tag="E", name=f"E{i}")
                    accEs[i] = acp.tile([128, 512], BF, tag="acc",
                                        name=f"acc{i}")
                return Es[i], accEs[i]

            def get_po(i):
                if i not in pos:
                    pos[i] = psoP.tile([128, 512], F32, tag="po", name=f"po{i}")
                return pos[i]

            state = {"deficit": 0.0, "drain": 0, "fillers": [], "fi": 0,
                     "fqc": None}

            def fill_one():
                if state["fi"] < len(state["fillers"]):
                    # during the last q-chunk the exp stream saturates ACT;
                    # keep its filler drains off that engine
                    de = 1 if state["fqc"] == TC - 2 else state["drain"]
                    o_tile(state["fqc"], state["fillers"][state["fi"]], de)
                    state["drain"] ^= 1
                    state["fi"] += 1
                    state["deficit"] -= 852.0
                    return True
                return False

            def drain_deficit(slack):
                while state["deficit"] > slack and fill_one():
                    pass

            def emit_s(i, p):
                qc, h = tasks[i]
                E, accE = get_e(i)
                s_pair(h, qc, p, 4 * (qc + 1), E, accE)
                state["deficit"] += 1140.0 - 426.0

            def emit_pv(i, p):
                qc, h = tasks[i]
                pv_pair(h, qc, p, Es[i], get_po(i), 4 * (qc + 1))
                state["deficit"] -= 426.0

            def emit_sums(i):
                qc, h = tasks[i]
                sums_b(h, qc, accEs[i], pos[i], nT_for[qc])

            # interleave q half-passes with qc0 attention tasks so the
            # exp latency of the smallest q-chunk hides under projections
            q_half(0, 0)
            q_half(0, 1)
            q_half(1, 0)
            q_half(1, 1)
            for p in range(npair_of(0)):
                emit_s(0, p)
            q_half(2, 0)
            for p in range(npair_of(1)):
                emit_s(1, p)
                emit_pv(0, p)
            emit_sums(0)
            q_half(2, 1)
            q_half(3, 0)
            for p in range(npair_of(2)):
                emit_s(2, p)
                emit_pv(1, p)
            emit_sums(1)
            q_half(3, 1)
            state["deficit"] = 0.0

            for i in range(3, NT + 1):
                if i < NT:
                    qc, h = tasks[i]
                    if h == 1 and qc > 0:
                        # O(qc-1) fillers become legal here: sums(qc-1, h3)
                        # was emitted at the end of the previous task, so the
                        # whole nT(qc-1) buffer has its writers queued.  Old
                        # leftovers must flush first (their nT buffer gets
                        # overwritten by sums(qc, h0) at the end of this
                        # task).
                        while fill_one():
                            pass
                        state["fillers"] = list(range(16))
                        state["fi"] = 0
                        state["fqc"] = qc - 1
                    for p in range(npair_of(i)):
                        emit_s(i, p)
                        if i >= 1 and p < npair_of(i - 1):
                            emit_pv(i - 1, p)
                        drain_deficit(426.0)
                else:
                    for p in range(npair_of(i - 1)):
                        emit_pv(i - 1, p)
                        drain_deficit(0.0)
                if i >= 1:
                    emit_sums(i - 1)

            # tail: O-projection for the last q-chunk
            while fill_one():
                pass
            for idx in range(14):
                o_tile(TC - 1, idx, idx & 1)
            for idx in (14, 15):
                o_tile(TC - 1, idx, 0, split_drain=True)

            psfP.release()
            psoP.release()
            ring1.release()

    nc.compile()
    _PROG["nc"] = nc
    return nc


def prepare_in_maps(x, Wq, bq, Wk, bk, Wv, bv, Wo, bo):
    bf = ml_dtypes.bfloat16

    def to_sb_layout(W):  # [E, cols] -> [128, ECH*cols] partition-major
        cols = W.shape[1]
        return np.ascontiguousarray(
            W.reshape(ECH, 128, cols).transpose(1, 0, 2).reshape(128, ECH * cols)
        ).astype(bf)

    in_maps = []
    for c in range(NCORES):
        b, g = c // 4, c % 4
        xt = np.ascontiguousarray(np.asarray(x[b]).T).astype(bf).reshape(
            ECH, 128, T
        )
        wq = np.stack(
            [
                to_sb_layout(np.asarray(Wq[:, g * 512 + j * D: g * 512 + (j + 1) * D]))
                for j in range(HPG)
            ],
            axis=0,
        ).reshape(HPG, 128, ECH * D)
        wk = to_sb_layout(np.asarray(Wk[:, g * D:(g + 1) * D]))
        wv = to_sb_layout(np.asarray(Wv[:, g * D:(g + 1) * D]))
        wo = np.ascontiguousarray(Wo[g * 512:(g + 1) * 512, :]).astype(bf).reshape(
            HPG, 128, EMBED
        )
        bqc = np.ascontiguousarray(
            bq[g * 512:(g + 1) * 512].reshape(HPG, 128).T
        ).astype(np.float32)
        bkc = np.asarray(bk[g * D:(g + 1) * D]).reshape(128, 1).astype(np.float32)
        bvc = np.asarray(bv[g * D:(g + 1) * D]).reshape(1, 128).astype(bf)
        in_maps.append(
            {
                "xt": xt,
                "wq": wq,
                "wk": wk,
                "wv": wv,
                "wo": wo,
                "bq": bqc,
                "bk": bkc,
                "bv": bvc,
            }
        )
    return in_maps


def combine_outputs(results, bo):
    out = np.empty((2, T, EMBED), dtype=np.float32)
    for b in range(2):
        acc = results[b * 4]["out"].astype(np.float32)
        for g in range(1, 4):
            acc += results[b * 4 + g]["out"].astype(np.float32)
        out[b] = acc + np.asarray(bo)[None, :].astype(np.float32)
    return out


def kernel(x, Wq, bq, Wk, bk, Wv, bv, Wo, bo):
    from concourse.bass_utils import run_bass_kernel_spmd

    nc = build_program()
    in_maps = prepare_in_maps(x, Wq, bq, Wk, bk, Wv, bv, Wo, bo)
    res = run_bass_kernel_spmd(nc, in_maps, list(range(NCORES)))
    return combine_outputs(res.results, np.asarray(bo))


# revision 5
# speedup vs baseline: 1.0309x; 1.0281x over previous
"""Grouped-Query Attention on 8 Trainium2 NeuronCores — v2.

Sharding: TP-4 over KV groups x DP-2 over batch.
Core c handles batch b = c // 4, group g = c % 4 (4 query heads, 1 KV group).

Differences vs v1:
  - V is projected directly into natural [t, d] layout in phase 1
    (stationary = xt tile, moving = Wv chunk) -- no PE transposes.
  - Softmax denominators: E accumulated over tk on DVE (bf16 2x), then a
    single allones[128,128] @ accE matmul broadcasts the denominator --
    removes 160 sum-matmuls + 16 broadcast-matmuls from PE.
  - Causal masking via Pool-engine affine_select (in-place on E).
  - Software-pipelined PE emission: S(h) pairs interleave with PV(h-1)
    pairs and O-projection(qc-1) filler matmuls.
  - DMA: wq split per head and interleaved with the xt stream; weights
    pre-transposed on host for 4KB-contiguous descriptors.
"""

import numpy as np
import ml_dtypes

EMBED = 2048
T = 2048
D = 128           # head dim
NQH = 16          # query heads
NG = 4            # kv groups
HPG = NQH // NG   # query heads per group = 4
NCORES = 8
ECH = EMBED // 128   # 16 contraction chunks
TC = T // 512        # 4 t-chunks of 512
TT = T // 128        # 16 t-tiles of 128
SCALE = 1.0 / float(np.sqrt(D))

_PROG = {}


def build_program():
    if "nc" in _PROG:
        return _PROG["nc"]

    from contextlib import ExitStack
    import concourse.mybir as mybir
    from concourse import bacc, tile

    # Drop redundant consecutive Ldweights with identical keys (the Tile
    # legalizer emits one per Matmult even when the stationary is unchanged).
    if not getattr(tile.tile_legalize, "_ldw_dedup", False):
        _orig_legalize = tile.tile_legalize

        def _dedup_legalize(ordered, nc_):
            ordered = _orig_legalize(ordered, nc_)
            for bb, insts in ordered.items():
                out = []
                state = None
                for inst in insts:
                    tn = type(inst).__name__
                    if tn == "InstLdweights":
                        key = (
                            str(inst.ins[0]),
                            str(getattr(inst, "is_transpose", None)),
                            str(getattr(inst, "tile_position", None)),
                            str(getattr(inst, "perf_mode", None)),
                        )
                        if key == state:
                            continue
                        state = key
                    out.append(inst)
                ordered[bb] = out
            return ordered

        _dedup_legalize._ldw_dedup = True
        tile.tile_legalize = _dedup_legalize

    dt = mybir.dt
    BF = dt.bfloat16
    F32 = dt.float32
    AF = mybir.ActivationFunctionType

    nc = bacc.Bacc("TRN2", target_bir_lowering=False, debug=False)

    xt_d = nc.dram_tensor("xt", [ECH, 128, T], BF, kind="ExternalInput")
    wq_d = nc.dram_tensor("wq", [HPG, 128, ECH * D], BF, kind="ExternalInput")
    wk_d = nc.dram_tensor("wk", [128, ECH * D], BF, kind="ExternalInput")
    wv_d = nc.dram_tensor("wv", [128, ECH * D], BF, kind="ExternalInput")
    wo_d = nc.dram_tensor("wo", [HPG, 128, EMBED], BF, kind="ExternalInput")
    bq_d = nc.dram_tensor("bq", [128, HPG], F32, kind="ExternalInput")
    bk_d = nc.dram_tensor("bk", [128, 1], F32, kind="ExternalInput")
    bv_d = nc.dram_tensor("bv", [1, 128], BF, kind="ExternalInput")
    out_d = nc.dram_tensor("out", [T, EMBED], BF, kind="ExternalOutput")

    with tile.TileContext(nc) as tc, ExitStack() as ctx:
        pers = ctx.enter_context(tc.tile_pool(name="pers", bufs=1))

        wq_sb = pers.tile([128, HPG, ECH * D], BF)
        wk_sb = pers.tile([128, ECH * D], BF)
        wv_sb = pers.tile([128, ECH * D], BF)
        wo_sb = pers.tile([128, HPG, EMBED], BF)
        bq_sb = pers.tile([128, HPG], F32)
        bk_sb = pers.tile([128, 1], F32)
        bv_sb = pers.tile([1, 128], BF)
        qT_sb = pers.tile([128, HPG, T], BF)
        kT_sb = pers.tile([128, T], BF)
        v_sb = pers.tile([128, TT, D], BF)
        nT_a = pers.tile([128, HPG, 512], BF)
        nT_b = pers.tile([128, HPG, 512], BF)
        ones1 = pers.tile([1, 128], BF)
        allones = pers.tile([128, 128], BF)

        nc.gpsimd.memset(ones1[:], 1.0)
        nc.gpsimd.memset(allones[:], 1.0)

        # ---- Phase 1: projections ----
        if True:
            xt_sb = pers.tile([128, ECH, T], BF)

            # All input DMAs on the sync queue in priority order (the DMA
            # engines are a shared serial resource): wk/wv + biases first,
            # then xt chunks with wq heads interleaved, wo last (needed only
            # by the O-projection ~60us later).
            nc.sync.dma_start(wk_sb[:, 0:D], wk_d[:, 0:D])
            nc.sync.dma_start(xt_sb[:, 0, 0:1024], xt_d[0][:, 0:1024])
            nc.sync.dma_start(wv_sb[:, 0:D], wv_d[:, 0:D])
            nc.sync.dma_start(xt_sb[:, 0, 1024:], xt_d[0][:, 1024:])
            nc.sync.dma_start(wk_sb[:, D:], wk_d[:, D:])
            nc.sync.dma_start(xt_sb[:, 1, :], xt_d[1])
            nc.sync.dma_start(wv_sb[:, D:], wv_d[:, D:])
            nc.sync.dma_start(xt_sb[:, 2, :], xt_d[2])
            nc.sync.dma_start(bk_sb[:], bk_d[:])
            nc.sync.dma_start(bv_sb[:], bv_d[:])
            nc.sync.dma_start(bq_sb[:], bq_d[:])
            wq_next = 0
            for ec in range(3, ECH):
                nc.sync.dma_start(xt_sb[:, ec, :], xt_d[ec])
                if ec % 6 == 2 and wq_next < HPG:
                    nc.sync.dma_start(wq_sb[:, wq_next, :], wq_d[wq_next])
                    wq_next += 1
            while wq_next < HPG:
                nc.sync.dma_start(wq_sb[:, wq_next, :], wq_d[wq_next])
                wq_next += 1
            nc.sync.dma_start(wo_sb[:], wo_d.ap().rearrange("h p e -> p h e"))

            # --- phase-1 PSUM: v accumulates on the right stack, kT on
            # the left.  kT's pool releases into ring1 (q half-passes + s2
            # pairs); v's pool releases into the po/pf pools.  No pool
            # barrier separates phase 1 from attention.
            kvB = tc.alloc_tile_pool(name="kvB", bufs=1, space="PSUM",
                                     side="right")
            vps = kvB.tile([128, TT, D], F32, tag="v", name="vps")
            kvA = tc.alloc_tile_pool(name="kvA", bufs=1, space="PSUM",
                                     side="left")
            kps = kvA.tile([128, T], F32, tag="k", name="kps")
            for ec in range(ECH):
                wkc = wk_sb[:, ec * D:(ec + 1) * D]
                for t5 in range(TC):
                    nc.tensor.matmul(
                        kps[:, t5 * 512:(t5 + 1) * 512],
                        wkc,
                        xt_sb[:, ec, t5 * 512:(t5 + 1) * 512],
                        start=(ec == 0),
                        stop=(ec == ECH - 1),
                    )
                wvc = wv_sb[:, ec * D:(ec + 1) * D]
                for tt in range(TT):
                    # 4 tt-tiles share a PSUM bank; `start` zeroes the whole
                    # 2KB bank region, so only the bank's first write starts
                    # the group and its last write stops it.
                    nc.tensor.matmul(
                        vps[:, tt, :],
                        xt_sb[:, ec, tt * D:(tt + 1) * D],
                        wvc,
                        start=(ec == 0 and tt % 4 == 0),
                        stop=False,
                    )
            # fold bv in as a rank-1 accumulation, closing each group
            for tt in range(TT):
                nc.tensor.matmul(
                    vps[:, tt, :], ones1[:], bv_sb[:],
                    start=False, stop=(tt % 4 == 3),
                )
            # drains: kT spread over ACT/DVE/Pool; v split DVE/Pool
            for t5 in range(TC):
                sl = slice(t5 * 512, (t5 + 1) * 512)
                nc.scalar.activation(
                    kT_sb[:, sl], kps[:, sl], AF.Identity, bias=bk_sb[:]
                )
            for tt in range(TT):
                nc.vector.tensor_copy(v_sb[:, tt, :], vps[:, tt, :])
            kvA.release()
            ring1 = tc.alloc_tile_pool(name="ring1", bufs=2, space="PSUM",
                                       side="left")
            kvB.release()
            psoP = tc.alloc_tile_pool(name="psoP", bufs=2, space="PSUM",
                                      side="right")
            psfP = tc.alloc_tile_pool(name="psfP", bufs=2, space="PSUM",
                                      side="right")

            def q_half(j, th):
                ps = ring1.tile([128, 1024], F32, tag="r", name=f"q{j}h{th}")
                for ec in range(ECH):
                    lhsT = wq_sb[:, j, ec * D:(ec + 1) * D]
                    for t5 in range(2):
                        lo = th * 1024 + t5 * 512
                        nc.tensor.matmul(
                            ps[:, t5 * 512:(t5 + 1) * 512],
                            lhsT,
                            xt_sb[:, ec, lo:lo + 512],
                            start=(ec == 0),
                            stop=(ec == ECH - 1),
                        )
                if th == 0:
                    nc.scalar.activation(
                        qT_sb[:, j, th * 1024:(th + 1) * 1024], ps[:],
                        AF.Identity, bias=bq_sb[:, j:j + 1],
                    )
                else:
                    nc.vector.tensor_scalar_add(
                        qT_sb[:, j, th * 1024:(th + 1) * 1024], ps[:],
                        bq_sb[:, j:j + 1],
                    )

        # ---- Phase 2: attention + O-projection, software-pipelined ----
        with (
            tc.tile_pool(name="eb", bufs=2) as ebp,
            tc.tile_pool(name="acp", bufs=2) as acp,
            tc.tile_pool(name="rcp", bufs=2) as rcp,
            tc.tile_pool(name="fsb", bufs=4) as fsb,
        ):
            nT_for = {}

            def dg_off(qc, tk):
                # columns [0, off) of tile tk are fully below the causal
                # diagonal (masked out) -- skip computing them entirely
                return max(0, (tk - 4 * qc)) * D if tk >= 4 * qc else 0

            def s_pair(h, qc, tkp, nk, E, accE):
                """Two S matmuls -> exp pair -> mask diag -> accumulate."""
                s2 = ring1.tile([128, 2, 512], F32, tag="r", name="s2t")
                # both tiles of a pair computed at the pair's min diagonal
                # offset so the exp reads a fully-written region
                woff = dg_off(qc, tkp * 2)
                for u in range(2):
                    tk = tkp * 2 + u
                    nc.tensor.matmul(
                        s2[:, u, woff:],
                        kT_sb[:, tk * D:(tk + 1) * D],
                        qT_sb[:, h, qc * 512 + woff:(qc + 1) * 512],
                        start=True,
                        stop=True,
                    )
                nc.scalar.activation(
                    E[:, tkp * 2:tkp * 2 + 2, woff:], s2[:, :, woff:],
                    AF.Exp, scale=SCALE
                )
                for u in range(2):
                    tk = tkp * 2 + u
                    off = dg_off(qc, tk)
                    if tk >= 4 * qc:
                        # zero E[p, woff+f] where woff + f < off + p
                        nc.gpsimd.affine_select(
                            out=E[:, tk, woff:],
                            in_=E[:, tk, woff:],
                            pattern=[[1, 512 - woff]],
                            compare_op=mybir.AluOpType.is_ge,
                            fill=0.0,
                            base=woff - off,
                            channel_multiplier=-1,
                        )
                    if tk == 0:
                        nc.vector.tensor_copy(accE[:], E[:, 0, :])
                    else:
                        nc.vector.tensor_add(
                            accE[:, woff:], accE[:, woff:], E[:, tk, woff:]
                        )

            def pv_pair(h, qc, tkp, E, po, nk):
                woff = dg_off(qc, tkp * 2)
                for u in range(2):
                    tk = tkp * 2 + u
                    nc.tensor.matmul(
                        po[:, woff:], v_sb[:, tk, :], E[:, tk, woff:],
                        start=(tk == 0), stop=(tk == nk - 1),
                    )

            def sums_b(h, qc, accE, po, nT):
                sumsB = psfP.tile([128, 512], F32, tag="pf", name="sumsBt")
                nc.tensor.matmul(
                    sumsB[:], allones[:], accE[:], start=True, stop=True
                )
                recipS = rcp.tile([128, 512], F32, tag="recip", name="recipSt")
                nc.vector.reciprocal(recipS[:], sumsB[:])
                nc.vector.tensor_mul(nT[:, h, :], po[:], recipS[:])

            def o_tile(qc_prev, idx, drain_eng, split_drain=False):
                """One O-projection output tile: 4 matmuls + copy + DMA."""
                qt, ecol = divmod(idx, 4)
                nT = nT_for[qc_prev]
                pf = psfP.tile([128, 512], F32, tag="pf", name="pft")
                for h in range(HPG):
                    nc.tensor.matmul(
                        pf[:],
                        nT[:, h, qt * 128:(qt + 1) * 128],
                        wo_sb[:, h, ecol * 512:(ecol + 1) * 512],
                        start=(h == 0),
                        stop=(h == HPG - 1),
                    )
                f_t = fsb.tile([128, 512], BF, tag="f", name="ftt")
                row = qc_prev * 4 + qt
                if split_drain:
                    # end-of-program: halve latency by draining on both
                    # engines and overlapping the two half DMAs
                    nc.scalar.activation(f_t[:, :256], pf[:, :256], AF.Identity)
                    nc.vector.tensor_copy(f_t[:, 256:], pf[:, 256:])
                    nc.sync.dma_start(
                        out_d[row * 128:(row + 1) * 128,
                              ecol * 512:ecol * 512 + 256],
                        f_t[:, :256],
                    )
                    nc.sync.dma_start(
                        out_d[row * 128:(row + 1) * 128,
                              ecol * 512 + 256:(ecol + 1) * 512],
                        f_t[:, 256:],
                    )
                    return
                if drain_eng == 0:
                    nc.scalar.activation(f_t[:], pf[:], AF.Identity)
                else:
                    nc.vector.tensor_copy(f_t[:], pf[:])
                nc.sync.dma_start(
                    out_d[row * 128:(row + 1) * 128,
                          ecol * 512:(ecol + 1) * 512],
                    f_t[:],
                )

            # Global depth-2 pipeline over tasks (qc, h): S(task i) pairs
            # interleave with PV(task i-2) pairs, with O(qc-1) filler tiles
            # inserted by a deficit model (ACT exp needs ~1140ns per pair vs
            # ~426ns of PE work per S or PV pair; an O tile is ~852ns).
            tasks = [(qc, h) for qc in range(TC) for h in range(HPG)]
            NT = len(tasks)
            npair_of = lambda i: 2 * (tasks[i][0] + 1)
            nT_for[0] = nT_a
            nT_for[1] = nT_b
            nT_for[2] = nT_a
            nT_for[3] = nT_b

            Es, accEs, pos = {}, {}, {}

            def get_e(i):
                if i not in Es:
                    Es[i] = ebp.tile([128, 16, 512], BF, tag="E", name=f"E{i}")
                    accEs[i] = acp.tile([128, 512], BF, tag="acc",
                                        name=f"acc{i}")
                return Es[i], accEs[i]

            def get_po(i):
                if i not in pos:
                    pos[i] = psoP.tile([128, 512], F32, tag="po", name=f"po{i}")
                return pos[i]

            state = {"deficit": 0.0, "drain": 0, "fillers": [], "fi": 0,
                     "fqc": None}

            def fill_one():
                if state["fi"] < len(state["fillers"]):
                    # during the last q-chunk the exp stream saturates ACT;
                    # keep its filler drains off that engine
                    de = 1 if state["fqc"] == TC - 2 else state["drain"]
                    o_tile(state["fqc"], state["fillers"][state["fi"]], de)
                    state["drain"] ^= 1
                    state["fi"] += 1
                    state["deficit"] -= 852.0
                    return True
                return False

            def drain_deficit(slack):
                while state["deficit"] > slack and fill_one():
                    pass

            def emit_s(i, p):
                qc, h = tasks[i]
                E, accE = get_e(i)
                s_pair(h, qc, p, 4 * (qc + 1), E, accE)
                state["deficit"] += 1340.0 - 426.0

            def emit_pv(i, p):
                qc, h = tasks[i]
                pv_pair(h, qc, p, Es[i], get_po(i), 4 * (qc + 1))
                state["deficit"] -= 426.0

            def emit_sums(i):
                qc, h = tasks[i]
                sums_b(h, qc, accEs[i], pos[i], nT_for[qc])

            # interleave q half-passes with qc0 attention tasks so the
            # exp latency of the smallest q-chunk hides under projections
            q_half(0, 0)
            q_half(0, 1)
            q_half(1, 0)
            q_half(1, 1)
            for p in range(npair_of(0)):
                emit_s(0, p)
            q_half(2, 0)
            for p in range(npair_of(1)):
                emit_s(1, p)
                emit_pv(0, p)
            emit_sums(0)
            q_half(2, 1)
            q_half(3, 0)
            for p in range(npair_of(2)):
                emit_s(2, p)
                emit_pv(1, p)
            emit_sums(1)
            q_half(3, 1)
            state["deficit"] = 0.0

            for i in range(3, NT + 1):
                if i < NT:
                    qc, h = tasks[i]
                    if h == 1 and qc > 0:
                        # O(qc-1) fillers become legal here: sums(qc-1, h3)
                        # was emitted at the end of the previous task, so the
                        # whole nT(qc-1) buffer has its writers queued.  Old
                        # leftovers must flush first (their nT buffer gets
                        # overwritten by sums(qc, h0) at the end of this
                        # task).
                        while fill_one():
                            pass
                        state["fillers"] = list(range(16))
                        state["fi"] = 0
                        state["fqc"] = qc - 1
                    for p in range(npair_of(i)):
                        emit_s(i, p)
                        if i >= 1 and p < npair_of(i - 1):
                            emit_pv(i - 1, p)
                        drain_deficit(852.0)
                else:
                    for p in range(npair_of(i - 1)):
                        emit_pv(i - 1, p)
                        drain_deficit(0.0)
                if i >= 1:
                    emit_sums(i - 1)

            # tail: O-projection for the last q-chunk
            while fill_one():
                pass
            for idx in range(14):
                o_tile(TC - 1, idx, idx & 1)
            for idx in (14, 15):
                o_tile(TC - 1, idx, 0, split_drain=True)

            psfP.release()
            psoP.release()
            ring1.release()

    nc.compile()
    _PROG["nc"] = nc
    return nc


def prepare_in_maps(x, Wq, bq, Wk, bk, Wv, bv, Wo, bo):
    bf = ml_dtypes.bfloat16

    def to_sb_layout(W):  # [E, cols] -> [128, ECH*cols] partition-major
        cols = W.shape[1]
        return np.ascontiguousarray(
            W.reshape(ECH, 128, cols).transpose(1, 0, 2).reshape(128, ECH * cols)
        ).astype(bf)

    in_maps = []
    for c in range(NCORES):
        b, g = c // 4, c % 4
        xt = np.ascontiguousarray(np.asarray(x[b]).T).astype(bf).reshape(
            ECH, 128, T
        )
        wq = np.stack(
            [
                to_sb_layout(np.asarray(Wq[:, g * 512 + j * D: g * 512 + (j + 1) * D]))
                for j in range(HPG)
            ],
            axis=0,
        ).reshape(HPG, 128, ECH * D)
        wk = to_sb_layout(np.asarray(Wk[:, g * D:(g + 1) * D]))
        wv = to_sb_layout(np.asarray(Wv[:, g * D:(g + 1) * D]))
        wo = np.ascontiguousarray(Wo[g * 512:(g + 1) * 512, :]).astype(bf).reshape(
            HPG, 128, EMBED
        )
        bqc = np.ascontiguousarray(
            bq[g * 512:(g + 1) * 512].reshape(HPG, 128).T
        ).astype(np.float32)
        bkc = np.asarray(bk[g * D:(g + 1) * D]).reshape(128, 1).astype(np.float32)
        bvc = np.asarray(bv[g * D:(g + 1) * D]).reshape(1, 128).astype(bf)
        in_maps.append(
            {
                "xt": xt,
                "wq": wq,
                "wk": wk,
                "wv": wv,
                "wo": wo,
                "bq": bqc,
                "bk": bkc,
                "bv": bvc,
            }
        )
    return in_maps


def combine_outputs(results, bo):
    out = np.empty((2, T, EMBED), dtype=np.float32)
    for b in range(2):
        acc = results[b * 4]["out"].astype(np.float32)
        for g in range(1, 4):
            acc += results[b * 4 + g]["out"].astype(np.float32)
        out[b] = acc + np.asarray(bo)[None, :].astype(np.float32)
    return out


def kernel(x, Wq, bq, Wk, bk, Wv, bv, Wo, bo):
    from concourse.bass_utils import run_bass_kernel_spmd

    nc = build_program()
    in_maps = prepare_in_maps(x, Wq, bq, Wk, bk, Wv, bv, Wo, bo)
    res = run_bass_kernel_spmd(nc, in_maps, list(range(NCORES)))
    return combine_outputs(res.results, np.asarray(bo))


# revision 6
# speedup vs baseline: 1.0405x; 1.0093x over previous
"""Grouped-Query Attention on 8 Trainium2 NeuronCores — v2.

Sharding: TP-4 over KV groups x DP-2 over batch.
Core c handles batch b = c // 4, group g = c % 4 (4 query heads, 1 KV group).

Differences vs v1:
  - V is projected directly into natural [t, d] layout in phase 1
    (stationary = xt tile, moving = Wv chunk) -- no PE transposes.
  - Softmax denominators: E accumulated over tk on DVE (bf16 2x), then a
    single allones[128,128] @ accE matmul broadcasts the denominator --
    removes 160 sum-matmuls + 16 broadcast-matmuls from PE.
  - Causal masking via Pool-engine affine_select (in-place on E).
  - Software-pipelined PE emission: S(h) pairs interleave with PV(h-1)
    pairs and O-projection(qc-1) filler matmuls.
  - DMA: wq split per head and interleaved with the xt stream; weights
    pre-transposed on host for 4KB-contiguous descriptors.
"""

import numpy as np
import ml_dtypes

EMBED = 2048
T = 2048
D = 128           # head dim
NQH = 16          # query heads
NG = 4            # kv groups
HPG = NQH // NG   # query heads per group = 4
NCORES = 8
ECH = EMBED // 128   # 16 contraction chunks
TC = T // 512        # 4 t-chunks of 512
TT = T // 128        # 16 t-tiles of 128
SCALE = 1.0 / float(np.sqrt(D))

_PROG = {}


def build_program():
    if "nc" in _PROG:
        return _PROG["nc"]

    from contextlib import ExitStack
    import concourse.mybir as mybir
    from concourse import bacc, tile

    # Drop redundant consecutive Ldweights with identical keys (the Tile
    # legalizer emits one per Matmult even when the stationary is unchanged).
    if not getattr(tile.tile_legalize, "_ldw_dedup", False):
        _orig_legalize = tile.tile_legalize

        def _dedup_legalize(ordered, nc_):
            ordered = _orig_legalize(ordered, nc_)
            for bb, insts in ordered.items():
                out = []
                state = None
                for inst in insts:
                    tn = type(inst).__name__
                    if tn == "InstLdweights":
                        key = (
                            str(inst.ins[0]),
                            str(getattr(inst, "is_transpose", None)),
                            str(getattr(inst, "tile_position", None)),
                            str(getattr(inst, "perf_mode", None)),
                        )
                        if key == state:
                            continue
                        state = key
                    out.append(inst)
                ordered[bb] = out
            return ordered

        _dedup_legalize._ldw_dedup = True
        tile.tile_legalize = _dedup_legalize

    dt = mybir.dt
    BF = dt.bfloat16
    F32 = dt.float32
    AF = mybir.ActivationFunctionType

    nc = bacc.Bacc("TRN2", target_bir_lowering=False, debug=False)

    xt_d = nc.dram_tensor("xt", [ECH, 128, T], BF, kind="ExternalInput")
    wq_d = nc.dram_tensor("wq", [HPG, 128, ECH * D], BF, kind="ExternalInput")
    wk_d = nc.dram_tensor("wk", [128, ECH * D], BF, kind="ExternalInput")
    wv_d = nc.dram_tensor("wv", [128, ECH * D], BF, kind="ExternalInput")
    wo_d = nc.dram_tensor("wo", [HPG, 128, EMBED], BF, kind="ExternalInput")
    bq_d = nc.dram_tensor("bq", [128, HPG], F32, kind="ExternalInput")
    bk_d = nc.dram_tensor("bk", [128, 1], F32, kind="ExternalInput")
    bv_d = nc.dram_tensor("bv", [1, 128], BF, kind="ExternalInput")
    out_d = nc.dram_tensor("out", [T, EMBED], BF, kind="ExternalOutput")

    with tile.TileContext(nc) as tc, ExitStack() as ctx:
        pers = ctx.enter_context(tc.tile_pool(name="pers", bufs=1))

        wq_sb = pers.tile([128, HPG, ECH * D], BF)
        wk_sb = pers.tile([128, ECH * D], BF)
        wv_sb = pers.tile([128, ECH * D], BF)
        wo_sb = pers.tile([128, HPG, EMBED], BF)
        bq_sb = pers.tile([128, HPG], F32)
        bk_sb = pers.tile([128, 1], F32)
        bv_sb = pers.tile([1, 128], BF)
        qT_sb = pers.tile([128, HPG, T], BF)
        kT_sb = pers.tile([128, T], BF)
        v_sb = pers.tile([128, TT, D], BF)
        nT_a = pers.tile([128, HPG, 512], BF)
        nT_b = pers.tile([128, HPG, 512], BF)
        ones1 = pers.tile([1, 128], BF)
        allones = pers.tile([128, 128], BF)

        nc.gpsimd.memset(ones1[:], 1.0)
        nc.gpsimd.memset(allones[:], 1.0)

        # ---- Phase 1: projections ----
        if True:
            xt_sb = pers.tile([128, ECH, T], BF)

            # All input DMAs on the sync queue in priority order (the DMA
            # engines are a shared serial resource): wk/wv + biases first,
            # then xt chunks with wq heads interleaved, wo last (needed only
            # by the O-projection ~60us later).
            nc.sync.dma_start(wk_sb[:, 0:D], wk_d[:, 0:D])
            nc.sync.dma_start(xt_sb[:, 0, 0:1024], xt_d[0][:, 0:1024])
            nc.sync.dma_start(wv_sb[:, 0:D], wv_d[:, 0:D])
            nc.sync.dma_start(xt_sb[:, 0, 1024:], xt_d[0][:, 1024:])
            nc.sync.dma_start(wk_sb[:, D:], wk_d[:, D:])
            nc.sync.dma_start(xt_sb[:, 1, :], xt_d[1])
            nc.sync.dma_start(wv_sb[:, D:], wv_d[:, D:])
            nc.sync.dma_start(xt_sb[:, 2, :], xt_d[2])
            nc.sync.dma_start(bk_sb[:], bk_d[:])
            nc.sync.dma_start(bv_sb[:], bv_d[:])
            nc.sync.dma_start(bq_sb[:], bq_d[:])
            wq_next = 0
            for ec in range(3, ECH):
                nc.sync.dma_start(xt_sb[:, ec, :], xt_d[ec])
                if ec % 6 == 2 and wq_next < HPG:
                    nc.sync.dma_start(wq_sb[:, wq_next, :], wq_d[wq_next])
                    wq_next += 1
            while wq_next < HPG:
                nc.sync.dma_start(wq_sb[:, wq_next, :], wq_d[wq_next])
                wq_next += 1
            nc.sync.dma_start(wo_sb[:], wo_d.ap().rearrange("h p e -> p h e"))

            # --- phase-1 PSUM: v accumulates on the right stack, kT on
            # the left.  kT's pool releases into ring1 (q half-passes + s2
            # pairs); v's pool releases into the po/pf pools.  No pool
            # barrier separates phase 1 from attention.
            kvB = tc.alloc_tile_pool(name="kvB", bufs=1, space="PSUM",
                                     side="right")
            vps = kvB.tile([128, TT, D], F32, tag="v", name="vps")
            kvA = tc.alloc_tile_pool(name="kvA", bufs=1, space="PSUM",
                                     side="left")
            kps = kvA.tile([128, T], F32, tag="k", name="kps")
            for ec in range(ECH):
                wkc = wk_sb[:, ec * D:(ec + 1) * D]
                for t5 in range(TC):
                    nc.tensor.matmul(
                        kps[:, t5 * 512:(t5 + 1) * 512],
                        wkc,
                        xt_sb[:, ec, t5 * 512:(t5 + 1) * 512],
                        start=(ec == 0),
                        stop=(ec == ECH - 1),
                    )
                wvc = wv_sb[:, ec * D:(ec + 1) * D]
                for tt in range(TT):
                    # 4 tt-tiles share a PSUM bank; `start` zeroes the whole
                    # 2KB bank region, so only the bank's first write starts
                    # the group and its last write stops it.
                    nc.tensor.matmul(
                        vps[:, tt, :],
                        xt_sb[:, ec, tt * D:(tt + 1) * D],
                        wvc,
                        start=(ec == 0 and tt % 4 == 0),
                        stop=False,
                    )
            # fold bv in as a rank-1 accumulation, closing each group
            for tt in range(TT):
                nc.tensor.matmul(
                    vps[:, tt, :], ones1[:], bv_sb[:],
                    start=False, stop=(tt % 4 == 3),
                )
            # drains: kT spread over ACT/DVE/Pool; v split DVE/Pool
            nc.scalar.activation(
                kT_sb[:], kps[:], AF.Identity, bias=bk_sb[:]
            )
            nc.vector.tensor_copy(v_sb[:], vps[:])
            kvA.release()
            ring1 = tc.alloc_tile_pool(name="ring1", bufs=2, space="PSUM",
                                       side="left")
            kvB.release()
            psoP = tc.alloc_tile_pool(name="psoP", bufs=2, space="PSUM",
                                      side="right")
            psfP = tc.alloc_tile_pool(name="psfP", bufs=2, space="PSUM",
                                      side="right")

            def q_half(j, th):
                ps = ring1.tile([128, 1024], F32, tag="r", name=f"q{j}h{th}")
                for ec in range(ECH):
                    lhsT = wq_sb[:, j, ec * D:(ec + 1) * D]
                    for t5 in range(2):
                        lo = th * 1024 + t5 * 512
                        nc.tensor.matmul(
                            ps[:, t5 * 512:(t5 + 1) * 512],
                            lhsT,
                            xt_sb[:, ec, lo:lo + 512],
                            start=(ec == 0),
                            stop=(ec == ECH - 1),
                        )
                if th == 0:
                    nc.scalar.activation(
                        qT_sb[:, j, th * 1024:(th + 1) * 1024], ps[:],
                        AF.Identity, bias=bq_sb[:, j:j + 1],
                    )
                else:
                    nc.vector.tensor_scalar_add(
                        qT_sb[:, j, th * 1024:(th + 1) * 1024], ps[:],
                        bq_sb[:, j:j + 1],
                    )

        # ---- Phase 2: attention + O-projection, software-pipelined ----
        with (
            tc.tile_pool(name="eb", bufs=2) as ebp,
            tc.tile_pool(name="acp", bufs=2) as acp,
            tc.tile_pool(name="rcp", bufs=2) as rcp,
            tc.tile_pool(name="fsb", bufs=4) as fsb,
        ):
            nT_for = {}

            def dg_off(qc, tk):
                # columns [0, off) of tile tk are fully below the causal
                # diagonal (masked out) -- skip computing them entirely
                return max(0, (tk - 4 * qc)) * D if tk >= 4 * qc else 0

            def s_pair(h, qc, tkp, nk, E, accE):
                """Two S matmuls -> exp pair -> mask diag -> accumulate."""
                s2 = ring1.tile([128, 2, 512], F32, tag="r", name="s2t")
                # both tiles of a pair computed at the pair's min diagonal
                # offset so the exp reads a fully-written region
                woff = dg_off(qc, tkp * 2)
                for u in range(2):
                    tk = tkp * 2 + u
                    nc.tensor.matmul(
                        s2[:, u, woff:],
                        kT_sb[:, tk * D:(tk + 1) * D],
                        qT_sb[:, h, qc * 512 + woff:(qc + 1) * 512],
                        start=True,
                        stop=True,
                    )
                nc.scalar.activation(
                    E[:, tkp * 2:tkp * 2 + 2, woff:], s2[:, :, woff:],
                    AF.Exp, scale=SCALE
                )
                for u in range(2):
                    tk = tkp * 2 + u
                    off = dg_off(qc, tk)
                    if tk >= 4 * qc:
                        # zero E[p, woff+f] where woff + f < off + p
                        nc.gpsimd.affine_select(
                            out=E[:, tk, woff:],
                            in_=E[:, tk, woff:],
                            pattern=[[1, 512 - woff]],
                            compare_op=mybir.AluOpType.is_ge,
                            fill=0.0,
                            base=woff - off,
                            channel_multiplier=-1,
                        )
                    if tk == 0:
                        nc.vector.tensor_copy(accE[:], E[:, 0, :])
                    else:
                        nc.vector.tensor_add(
                            accE[:, woff:], accE[:, woff:], E[:, tk, woff:]
                        )

            def pv_pair(h, qc, tkp, E, po, nk):
                woff = dg_off(qc, tkp * 2)
                for u in range(2):
                    tk = tkp * 2 + u
                    nc.tensor.matmul(
                        po[:, woff:], v_sb[:, tk, :], E[:, tk, woff:],
                        start=(tk == 0), stop=(tk == nk - 1),
                    )

            def sums_b(h, qc, accE, po, nT):
                sumsB = psfP.tile([128, 512], F32, tag="pf", name="sumsBt")
                nc.tensor.matmul(
                    sumsB[:], allones[:], accE[:], start=True, stop=True
                )
                recipS = rcp.tile([128, 512], F32, tag="recip", name="recipSt")
                nc.vector.reciprocal(recipS[:], sumsB[:])
                nc.vector.tensor_mul(nT[:, h, :], po[:], recipS[:])

            def o_tile(qc_prev, idx, drain_eng, split_drain=False,
                       alt_q=False):
                """One O-projection output tile: 4 matmuls + copy + DMA."""
                qt, ecol = divmod(idx, 4)
                nT = nT_for[qc_prev]
                pf = psfP.tile([128, 512], F32, tag="pf", name="pft")
                for h in range(HPG):
                    nc.tensor.matmul(
                        pf[:],
                        nT[:, h, qt * 128:(qt + 1) * 128],
                        wo_sb[:, h, ecol * 512:(ecol + 1) * 512],
                        start=(h == 0),
                        stop=(h == HPG - 1),
                    )
                f_t = fsb.tile([128, 512], BF, tag="f", name="ftt")
                row = qc_prev * 4 + qt
                if split_drain:
                    # end-of-program: halve latency by draining on both
                    # engines and overlapping the two half DMAs
                    nc.scalar.activation(f_t[:, :256], pf[:, :256], AF.Identity)
                    nc.vector.tensor_copy(f_t[:, 256:], pf[:, 256:])
                    nc.sync.dma_start(
                        out_d[row * 128:(row + 1) * 128,
                              ecol * 512:ecol * 512 + 256],
                        f_t[:, :256],
                    )
                    nc.sync.dma_start(
                        out_d[row * 128:(row + 1) * 128,
                              ecol * 512 + 256:(ecol + 1) * 512],
                        f_t[:, 256:],
                    )
                    return
                if drain_eng == 0:
                    nc.scalar.activation(f_t[:], pf[:], AF.Identity)
                else:
                    nc.vector.tensor_copy(f_t[:], pf[:])
                dq = nc.scalar if alt_q else nc.sync
                dq.dma_start(
                    out_d[row * 128:(row + 1) * 128,
                          ecol * 512:(ecol + 1) * 512],
                    f_t[:],
                )

            # Global depth-2 pipeline over tasks (qc, h): S(task i) pairs
            # interleave with PV(task i-2) pairs, with O(qc-1) filler tiles
            # inserted by a deficit model (ACT exp needs ~1140ns per pair vs
            # ~426ns of PE work per S or PV pair; an O tile is ~852ns).
            tasks = [(qc, h) for qc in range(TC) for h in range(HPG)]
            NT = len(tasks)
            npair_of = lambda i: 2 * (tasks[i][0] + 1)
            nT_for[0] = nT_a
            nT_for[1] = nT_b
            nT_for[2] = nT_a
            nT_for[3] = nT_b

            Es, accEs, pos = {}, {}, {}

            def get_e(i):
                if i not in Es:
                    Es[i] = ebp.tile([128, 16, 512], BF, tag="E", name=f"E{i}")
                    accEs[i] = acp.tile([128, 512], BF, tag="acc",
                                        name=f"acc{i}")
                return Es[i], accEs[i]

            def get_po(i):
                if i not in pos:
                    pos[i] = psoP.tile([128, 512], F32, tag="po", name=f"po{i}")
                return pos[i]

            state = {"deficit": 0.0, "drain": 0, "fillers": [], "fi": 0,
                     "fqc": None}

            def fill_one():
                if state["fi"] < len(state["fillers"]):
                    # during the last q-chunk the exp stream saturates ACT;
                    # keep its filler drains off that engine
                    de = 1 if state["fqc"] == TC - 2 else state["drain"]
                    o_tile(state["fqc"], state["fillers"][state["fi"]], de)
                    state["drain"] ^= 1
                    state["fi"] += 1
                    state["deficit"] -= 852.0
                    return True
                return False

            def drain_deficit(slack):
                while state["deficit"] > slack and fill_one():
                    pass

            def emit_s(i, p):
                qc, h = tasks[i]
                E, accE = get_e(i)
                s_pair(h, qc, p, 4 * (qc + 1), E, accE)
                state["deficit"] += 1340.0 - 426.0

            def emit_pv(i, p):
                qc, h = tasks[i]
                pv_pair(h, qc, p, Es[i], get_po(i), 4 * (qc + 1))
                state["deficit"] -= 426.0

            def emit_sums(i):
                qc, h = tasks[i]
                sums_b(h, qc, accEs[i], pos[i], nT_for[qc])

            # interleave q half-passes with qc0 attention tasks so the
            # exp latency of the smallest q-chunk hides under projections
            q_half(0, 0)
            q_half(0, 1)
            q_half(1, 0)
            q_half(1, 1)
            for p in range(npair_of(0)):
                emit_s(0, p)
            q_half(2, 0)
            for p in range(npair_of(1)):
                emit_s(1, p)
                emit_pv(0, p)
            emit_sums(0)
            q_half(2, 1)
            q_half(3, 0)
            for p in range(npair_of(2)):
                emit_s(2, p)
                emit_pv(1, p)
            emit_sums(1)
            q_half(3, 1)
            state["deficit"] = 0.0

            for i in range(3, NT + 1):
                if i < NT:
                    qc, h = tasks[i]
                    if h == 1 and qc > 0:
                        # O(qc-1) fillers become legal here: sums(qc-1, h3)
                        # was emitted at the end of the previous task, so the
                        # whole nT(qc-1) buffer has its writers queued.  Old
                        # leftovers must flush first (their nT buffer gets
                        # overwritten by sums(qc, h0) at the end of this
                        # task).
                        while fill_one():
                            pass
                        state["fillers"] = list(range(16))
                        state["fi"] = 0
                        state["fqc"] = qc - 1
                    for p in range(npair_of(i)):
                        emit_s(i, p)
                        if i >= 1 and p < npair_of(i - 1):
                            emit_pv(i - 1, p)
                        drain_deficit(852.0)
                else:
                    for p in range(npair_of(i - 1)):
                        emit_pv(i - 1, p)
                        drain_deficit(0.0)
                if i >= 1:
                    emit_sums(i - 1)

            # tail: O-projection for the last q-chunk
            while fill_one():
                pass
            for idx in range(12):
                o_tile(TC - 1, idx, idx & 1)
            for idx in range(12, 16):
                o_tile(TC - 1, idx, idx & 1, alt_q=(idx & 1 == 0))

            psfP.release()
            psoP.release()
            ring1.release()

    nc.compile()
    _PROG["nc"] = nc
    return nc


def prepare_in_maps(x, Wq, bq, Wk, bk, Wv, bv, Wo, bo):
    bf = ml_dtypes.bfloat16

    def to_sb_layout(W):  # [E, cols] -> [128, ECH*cols] partition-major
        cols = W.shape[1]
        return np.ascontiguousarray(
            W.reshape(ECH, 128, cols).transpose(1, 0, 2).reshape(128, ECH * cols)
        ).astype(bf)

    in_maps = []
    for c in range(NCORES):
        b, g = c // 4, c % 4
        xt = np.ascontiguousarray(np.asarray(x[b]).T).astype(bf).reshape(
            ECH, 128, T
        )
        wq = np.stack(
            [
                to_sb_layout(np.asarray(Wq[:, g * 512 + j * D: g * 512 + (j + 1) * D]))
                for j in range(HPG)
            ],
            axis=0,
        ).reshape(HPG, 128, ECH * D)
        wk = to_sb_layout(np.asarray(Wk[:, g * D:(g + 1) * D]))
        wv = to_sb_layout(np.asarray(Wv[:, g * D:(g + 1) * D]))
        wo = np.ascontiguousarray(Wo[g * 512:(g + 1) * 512, :]).astype(bf).reshape(
            HPG, 128, EMBED
        )
        bqc = np.ascontiguousarray(
            bq[g * 512:(g + 1) * 512].reshape(HPG, 128).T
        ).astype(np.float32)
        bkc = np.asarray(bk[g * D:(g + 1) * D]).reshape(128, 1).astype(np.float32)
        bvc = np.asarray(bv[g * D:(g + 1) * D]).reshape(1, 128).astype(bf)
        in_maps.append(
            {
                "xt": xt,
                "wq": wq,
                "wk": wk,
                "wv": wv,
                "wo": wo,
                "bq": bqc,
                "bk": bkc,
                "bv": bvc,
            }
        )
    return in_maps


def combine_outputs(results, bo):
    out = np.empty((2, T, EMBED), dtype=np.float32)
    for b in range(2):
        acc = results[b * 4]["out"].astype(np.float32)
        for g in range(1, 4):
            acc += results[b * 4 + g]["out"].astype(np.float32)
        out[b] = acc + np.asarray(bo)[None, :].astype(np.float32)
    return out


def kernel(x, Wq, bq, Wk, bk, Wv, bv, Wo, bo):
    from concourse.bass_utils import run_bass_kernel_spmd

    nc = build_program()
    in_maps = prepare_in_maps(x, Wq, bq, Wk, bk, Wv, bv, Wo, bo)
    res = run_bass_kernel_spmd(nc, in_maps, list(range(NCORES)))
    return combine_outputs(res.results, np.asarray(bo))


# revision 7
# speedup vs baseline: 1.0411x; 1.0005x over previous
"""Grouped-Query Attention on 8 Trainium2 NeuronCores — v2.

Sharding: TP-4 over KV groups x DP-2 over batch.
Core c handles batch b = c // 4, group g = c % 4 (4 query heads, 1 KV group).

Differences vs v1:
  - V is projected directly into natural [t, d] layout in phase 1
    (stationary = xt tile, moving = Wv chunk) -- no PE transposes.
  - Softmax denominators: E accumulated over tk on DVE (bf16 2x), then a
    single allones[128,128] @ accE matmul broadcasts the denominator --
    removes 160 sum-matmuls + 16 broadcast-matmuls from PE.
  - Causal masking via Pool-engine affine_select (in-place on E).
  - Software-pipelined PE emission: S(h) pairs interleave with PV(h-1)
    pairs and O-projection(qc-1) filler matmuls.
  - DMA: wq split per head and interleaved with the xt stream; weights
    pre-transposed on host for 4KB-contiguous descriptors.
"""

import numpy as np
import ml_dtypes

EMBED = 2048
T = 2048
D = 128           # head dim
NQH = 16          # query heads
NG = 4            # kv groups
HPG = NQH // NG   # query heads per group = 4
NCORES = 8
ECH = EMBED // 128   # 16 contraction chunks
TC = T // 512        # 4 t-chunks of 512
TT = T // 128        # 16 t-tiles of 128
SCALE = 1.0 / float(np.sqrt(D))

_PROG = {}


def build_program():
    if "nc" in _PROG:
        return _PROG["nc"]

    from contextlib import ExitStack
    import concourse.mybir as mybir
    from concourse import bacc, tile

    # Drop redundant consecutive Ldweights with identical keys (the Tile
    # legalizer emits one per Matmult even when the stationary is unchanged).
    if not getattr(tile.tile_legalize, "_ldw_dedup", False):
        _orig_legalize = tile.tile_legalize

        def _dedup_legalize(ordered, nc_):
            ordered = _orig_legalize(ordered, nc_)
            for bb, insts in ordered.items():
                out = []
                state = None
                for inst in insts:
                    tn = type(inst).__name__
                    if tn == "InstLdweights":
                        key = (
                            str(inst.ins[0]),
                            str(getattr(inst, "is_transpose", None)),
                            str(getattr(inst, "tile_position", None)),
                            str(getattr(inst, "perf_mode", None)),
                        )
                        if key == state:
                            continue
                        state = key
                    out.append(inst)
                ordered[bb] = out
            return ordered

        _dedup_legalize._ldw_dedup = True
        tile.tile_legalize = _dedup_legalize

    dt = mybir.dt
    BF = dt.bfloat16
    F32 = dt.float32
    AF = mybir.ActivationFunctionType

    nc = bacc.Bacc("TRN2", target_bir_lowering=False, debug=False)

    xt_d = nc.dram_tensor("xt", [ECH, 128, T], BF, kind="ExternalInput")
    wq_d = nc.dram_tensor("wq", [HPG, 128, ECH * D], BF, kind="ExternalInput")
    wk_d = nc.dram_tensor("wk", [128, ECH * D], BF, kind="ExternalInput")
    wv_d = nc.dram_tensor("wv", [128, ECH * D], BF, kind="ExternalInput")
    wo_d = nc.dram_tensor("wo", [HPG, 128, EMBED], BF, kind="ExternalInput")
    bq_d = nc.dram_tensor("bq", [128, HPG], F32, kind="ExternalInput")
    bk_d = nc.dram_tensor("bk", [128, 1], F32, kind="ExternalInput")
    bv_d = nc.dram_tensor("bv", [1, 128], BF, kind="ExternalInput")
    out_d = nc.dram_tensor("out", [T, EMBED], BF, kind="ExternalOutput")

    with tile.TileContext(nc) as tc, ExitStack() as ctx:
        pers = ctx.enter_context(tc.tile_pool(name="pers", bufs=1))

        wq_sb = pers.tile([128, HPG, ECH * D], BF)
        wk_sb = pers.tile([128, ECH * D], BF)
        wv_sb = pers.tile([128, ECH * D], BF)
        wo_sb = pers.tile([128, HPG, EMBED], BF)
        bq_sb = pers.tile([128, HPG], F32)
        bk_sb = pers.tile([128, 1], F32)
        bv_sb = pers.tile([1, 128], BF)
        qT_sb = pers.tile([128, HPG, T], BF)
        kT_sb = pers.tile([128, T], BF)
        v_sb = pers.tile([128, TT, D], BF)
        nT_a = pers.tile([128, HPG, 512], BF)
        nT_b = pers.tile([128, HPG, 512], BF)
        ones1 = pers.tile([1, 128], BF)
        allones = pers.tile([128, 128], BF)

        nc.gpsimd.memset(ones1[:], 1.0)
        nc.gpsimd.memset(allones[:], 1.0)

        # ---- Phase 1: projections ----
        if True:
            xt_sb = pers.tile([128, ECH, T], BF)

            # All input DMAs on the sync queue in priority order (the DMA
            # engines are a shared serial resource): wk/wv + biases first,
            # then xt chunks with wq heads interleaved, wo last (needed only
            # by the O-projection ~60us later).
            nc.sync.dma_start(wk_sb[:, 0:D], wk_d[:, 0:D])
            nc.sync.dma_start(xt_sb[:, 0, 0:1024], xt_d[0][:, 0:1024])
            nc.sync.dma_start(xt_sb[:, 0, 1024:], xt_d[0][:, 1024:])
            nc.sync.dma_start(wk_sb[:, D:2 * D], wk_d[:, D:2 * D])
            nc.sync.dma_start(xt_sb[:, 1, :], xt_d[1])
            nc.sync.dma_start(wv_sb[:, 0:D], wv_d[:, 0:D])
            nc.sync.dma_start(wk_sb[:, 2 * D:], wk_d[:, 2 * D:])
            nc.sync.dma_start(xt_sb[:, 2, :], xt_d[2])
            nc.sync.dma_start(wv_sb[:, D:], wv_d[:, D:])
            nc.sync.dma_start(bk_sb[:], bk_d[:])
            nc.sync.dma_start(bv_sb[:], bv_d[:])
            nc.sync.dma_start(bq_sb[:], bq_d[:])
            wq_next = 0
            for ec in range(3, ECH):
                nc.sync.dma_start(xt_sb[:, ec, :], xt_d[ec])
                if ec % 6 == 2 and wq_next < HPG:
                    nc.sync.dma_start(wq_sb[:, wq_next, :], wq_d[wq_next])
                    wq_next += 1
            while wq_next < HPG:
                nc.sync.dma_start(wq_sb[:, wq_next, :], wq_d[wq_next])
                wq_next += 1
            nc.sync.dma_start(wo_sb[:], wo_d.ap().rearrange("h p e -> p h e"))

            # --- phase-1 PSUM: v accumulates on the right stack, kT on
            # the left.  kT's pool releases into ring1 (q half-passes + s2
            # pairs); v's pool releases into the po/pf pools.  No pool
            # barrier separates phase 1 from attention.
            kvB = tc.alloc_tile_pool(name="kvB", bufs=1, space="PSUM",
                                     side="right")
            vps = kvB.tile([128, TT, D], F32, tag="v", name="vps")
            kvA = tc.alloc_tile_pool(name="kvA", bufs=1, space="PSUM",
                                     side="left")
            kps = kvA.tile([128, T], F32, tag="k", name="kps")
            def v_chunk(ec):
                wvc = wv_sb[:, ec * D:(ec + 1) * D]
                for tt in range(TT):
                    # 4 tt-tiles share a PSUM bank; `start` zeroes the whole
                    # 2KB bank region, so only the bank's first write starts
                    # the group and its last write stops it.
                    nc.tensor.matmul(
                        vps[:, tt, :],
                        xt_sb[:, ec, tt * D:(tt + 1) * D],
                        wvc,
                        start=(ec == 0 and tt % 4 == 0),
                        stop=False,
                    )

            # v lags k by one chunk so the early PE work needs a thinner
            # DMA prefix (wv can land after the first two xt chunks)
            for ec in range(ECH):
                wkc = wk_sb[:, ec * D:(ec + 1) * D]
                for t5 in range(TC):
                    nc.tensor.matmul(
                        kps[:, t5 * 512:(t5 + 1) * 512],
                        wkc,
                        xt_sb[:, ec, t5 * 512:(t5 + 1) * 512],
                        start=(ec == 0),
                        stop=(ec == ECH - 1),
                    )
                if ec >= 1:
                    v_chunk(ec - 1)
            v_chunk(ECH - 1)
            # fold bv in as a rank-1 accumulation, closing each group
            for tt in range(TT):
                nc.tensor.matmul(
                    vps[:, tt, :], ones1[:], bv_sb[:],
                    start=False, stop=(tt % 4 == 3),
                )
            # drains: kT spread over ACT/DVE/Pool; v split DVE/Pool
            nc.scalar.activation(
                kT_sb[:], kps[:], AF.Identity, bias=bk_sb[:]
            )
            nc.vector.tensor_copy(v_sb[:], vps[:])
            kvA.release()
            ring1 = tc.alloc_tile_pool(name="ring1", bufs=2, space="PSUM",
                                       side="left")
            kvB.release()
            psoP = tc.alloc_tile_pool(name="psoP", bufs=2, space="PSUM",
                                      side="right")
            psfP = tc.alloc_tile_pool(name="psfP", bufs=2, space="PSUM",
                                      side="right")

            def q_half(j, th):
                ps = ring1.tile([128, 1024], F32, tag="r", name=f"q{j}h{th}")
                for ec in range(ECH):
                    lhsT = wq_sb[:, j, ec * D:(ec + 1) * D]
                    for t5 in range(2):
                        lo = th * 1024 + t5 * 512
                        nc.tensor.matmul(
                            ps[:, t5 * 512:(t5 + 1) * 512],
                            lhsT,
                            xt_sb[:, ec, lo:lo + 512],
                            start=(ec == 0),
                            stop=(ec == ECH - 1),
                        )
                if th == 1:
                    nc.scalar.activation(
                        qT_sb[:, j, th * 1024:(th + 1) * 1024], ps[:],
                        AF.Identity, bias=bq_sb[:, j:j + 1],
                    )
                else:
                    nc.vector.tensor_scalar_add(
                        qT_sb[:, j, th * 1024:(th + 1) * 1024], ps[:],
                        bq_sb[:, j:j + 1],
                    )

        # ---- Phase 2: attention + O-projection, software-pipelined ----
        with (
            tc.tile_pool(name="eb", bufs=2) as ebp,
            tc.tile_pool(name="acp", bufs=2) as acp,
            tc.tile_pool(name="rcp", bufs=2) as rcp,
            tc.tile_pool(name="fsb", bufs=4) as fsb,
        ):
            nT_for = {}

            def dg_off(qc, tk):
                # columns [0, off) of tile tk are fully below the causal
                # diagonal (masked out) -- skip computing them entirely
                return max(0, (tk - 4 * qc)) * D if tk >= 4 * qc else 0

            def s_pair(h, qc, tkp, nk, E, accE):
                """Two S matmuls -> exp pair -> mask diag -> accumulate."""
                s2 = ring1.tile([128, 2, 512], F32, tag="r", name="s2t")
                # both tiles of a pair computed at the pair's min diagonal
                # offset so the exp reads a fully-written region
                woff = dg_off(qc, tkp * 2)
                for u in range(2):
                    tk = tkp * 2 + u
                    nc.tensor.matmul(
                        s2[:, u, woff:],
                        kT_sb[:, tk * D:(tk + 1) * D],
                        qT_sb[:, h, qc * 512 + woff:(qc + 1) * 512],
                        start=True,
                        stop=True,
                    )
                nc.scalar.activation(
                    E[:, tkp * 2:tkp * 2 + 2, woff:], s2[:, :, woff:],
                    AF.Exp, scale=SCALE
                )
                for u in range(2):
                    tk = tkp * 2 + u
                    off = dg_off(qc, tk)
                    if tk >= 4 * qc:
                        # zero E[p, woff+f] where woff + f < off + p
                        nc.gpsimd.affine_select(
                            out=E[:, tk, woff:],
                            in_=E[:, tk, woff:],
                            pattern=[[1, 512 - woff]],
                            compare_op=mybir.AluOpType.is_ge,
                            fill=0.0,
                            base=woff - off,
                            channel_multiplier=-1,
                        )
                    if tk == 0:
                        nc.vector.tensor_copy(accE[:], E[:, 0, :])
                    else:
                        nc.vector.tensor_add(
                            accE[:, woff:], accE[:, woff:], E[:, tk, woff:]
                        )

            def pv_pair(h, qc, tkp, E, po, nk):
                woff = dg_off(qc, tkp * 2)
                for u in range(2):
                    tk = tkp * 2 + u
                    nc.tensor.matmul(
                        po[:, woff:], v_sb[:, tk, :], E[:, tk, woff:],
                        start=(tk == 0), stop=(tk == nk - 1),
                    )

            def sums_b(h, qc, accE, po, nT):
                sumsB = psfP.tile([128, 512], F32, tag="pf", name="sumsBt")
                nc.tensor.matmul(
                    sumsB[:], allones[:], accE[:], start=True, stop=True
                )
                recipS = rcp.tile([128, 512], F32, tag="recip", name="recipSt")
                nc.vector.reciprocal(recipS[:], sumsB[:])
                nc.vector.tensor_mul(nT[:, h, :], po[:], recipS[:])

            def o_tile(qc_prev, idx, drain_eng, split_drain=False,
                       alt_q=False):
                """One O-projection output tile: 4 matmuls + copy + DMA."""
                qt, ecol = divmod(idx, 4)
                nT = nT_for[qc_prev]
                pf = psfP.tile([128, 512], F32, tag="pf", name="pft")
                for h in range(HPG):
                    nc.tensor.matmul(
                        pf[:],
                        nT[:, h, qt * 128:(qt + 1) * 128],
                        wo_sb[:, h, ecol * 512:(ecol + 1) * 512],
                        start=(h == 0),
                        stop=(h == HPG - 1),
                    )
                f_t = fsb.tile([128, 512], BF, tag="f", name="ftt")
                row = qc_prev * 4 + qt
                if split_drain:
                    # end-of-program: halve latency by draining on both
                    # engines and overlapping the two half DMAs
                    nc.scalar.activation(f_t[:, :256], pf[:, :256], AF.Identity)
                    nc.vector.tensor_copy(f_t[:, 256:], pf[:, 256:])
                    nc.sync.dma_start(
                        out_d[row * 128:(row + 1) * 128,
                              ecol * 512:ecol * 512 + 256],
                        f_t[:, :256],
                    )
                    nc.sync.dma_start(
                        out_d[row * 128:(row + 1) * 128,
                              ecol * 512 + 256:(ecol + 1) * 512],
                        f_t[:, 256:],
                    )
                    return
                if drain_eng == 0:
                    nc.scalar.activation(f_t[:], pf[:], AF.Identity)
                else:
                    nc.vector.tensor_copy(f_t[:], pf[:])
                dq = nc.scalar if alt_q else nc.sync
                dq.dma_start(
                    out_d[row * 128:(row + 1) * 128,
                          ecol * 512:(ecol + 1) * 512],
                    f_t[:],
                )

            # Global depth-2 pipeline over tasks (qc, h): S(task i) pairs
            # interleave with PV(task i-2) pairs, with O(qc-1) filler tiles
            # inserted by a deficit model (ACT exp needs ~1140ns per pair vs
            # ~426ns of PE work per S or PV pair; an O tile is ~852ns).
            tasks = [(qc, h) for qc in range(TC) for h in range(HPG)]
            NT = len(tasks)
            npair_of = lambda i: 2 * (tasks[i][0] + 1)
            nT_for[0] = nT_a
            nT_for[1] = nT_b
            nT_for[2] = nT_a
            nT_for[3] = nT_b

            Es, accEs, pos = {}, {}, {}

            def get_e(i):
                if i not in Es:
                    Es[i] = ebp.tile([128, 16, 512], BF, tag="E", name=f"E{i}")
                    accEs[i] = acp.tile([128, 512], BF, tag="acc",
                                        name=f"acc{i}")
                return Es[i], accEs[i]

            def get_po(i):
                if i not in pos:
                    pos[i] = psoP.tile([128, 512], F32, tag="po", name=f"po{i}")
                return pos[i]

            state = {"deficit": 0.0, "drain": 0, "fillers": [], "fi": 0,
                     "fqc": None}

            def fill_one():
                if state["fi"] < len(state["fillers"]):
                    # during the last q-chunk the exp stream saturates ACT;
                    # keep its filler drains off that engine
                    de = 1 if state["fqc"] == TC - 2 else state["drain"]
                    o_tile(state["fqc"], state["fillers"][state["fi"]], de)
                    state["drain"] ^= 1
                    state["fi"] += 1
                    state["deficit"] -= 852.0
                    return True
                return False

            def drain_deficit(slack):
                while state["deficit"] > slack and fill_one():
                    pass

            def emit_s(i, p):
                qc, h = tasks[i]
                E, accE = get_e(i)
                s_pair(h, qc, p, 4 * (qc + 1), E, accE)
                state["deficit"] += 1340.0 - 426.0

            def emit_pv(i, p):
                qc, h = tasks[i]
                pv_pair(h, qc, p, Es[i], get_po(i), 4 * (qc + 1))
                state["deficit"] -= 426.0

            def emit_sums(i):
                qc, h = tasks[i]
                sums_b(h, qc, accEs[i], pos[i], nT_for[qc])

            # interleave q half-passes with qc0 attention tasks so the
            # exp latency of the smallest q-chunk hides under projections
            q_half(0, 0)
            q_half(0, 1)
            q_half(1, 0)
            q_half(1, 1)
            for p in range(npair_of(0)):
                emit_s(0, p)
            q_half(2, 0)
            for p in range(npair_of(1)):
                emit_s(1, p)
                emit_pv(0, p)
            emit_sums(0)
            q_half(2, 1)
            q_half(3, 0)
            for p in range(npair_of(2)):
                emit_s(2, p)
                emit_pv(1, p)
            emit_sums(1)
            q_half(3, 1)
            state["deficit"] = 0.0

            for i in range(3, NT + 1):
                if i < NT:
                    qc, h = tasks[i]
                    if h == 1 and qc > 0:
                        # O(qc-1) fillers become legal here: sums(qc-1, h3)
                        # was emitted at the end of the previous task, so the
                        # whole nT(qc-1) buffer has its writers queued.  Old
                        # leftovers must flush first (their nT buffer gets
                        # overwritten by sums(qc, h0) at the end of this
                        # task).
                        while fill_one():
                            pass
                        state["fillers"] = list(range(16))
                        state["fi"] = 0
                        state["fqc"] = qc - 1
                    for p in range(npair_of(i)):
                        emit_s(i, p)
                        if i >= 1 and p < npair_of(i - 1):
                            emit_pv(i - 1, p)
                        drain_deficit(852.0)
                else:
                    for p in range(npair_of(i - 1)):
                        emit_pv(i - 1, p)
                        drain_deficit(0.0)
                if i >= 1:
                    emit_sums(i - 1)

            # tail: O-projection for the last q-chunk
            while fill_one():
                pass
            for idx in range(12):
                o_tile(TC - 1, idx, idx & 1)
            for idx in range(12, 16):
                o_tile(TC - 1, idx, idx & 1, alt_q=(idx & 1 == 0))

            psfP.release()
            psoP.release()
            ring1.release()

    nc.compile()
    _PROG["nc"] = nc
    return nc


def prepare_in_maps(x, Wq, bq, Wk, bk, Wv, bv, Wo, bo):
    bf = ml_dtypes.bfloat16

    def to_sb_layout(W):  # [E, cols] -> [128, ECH*cols] partition-major
        cols = W.shape[1]
        return np.ascontiguousarray(
            W.reshape(ECH, 128, cols).transpose(1, 0, 2).reshape(128, ECH * cols)
        ).astype(bf)

    in_maps = []
    for c in range(NCORES):
        b, g = c // 4, c % 4
        xt = np.ascontiguousarray(np.asarray(x[b]).T).astype(bf).reshape(
            ECH, 128, T
        )
        wq = np.stack(
            [
                to_sb_layout(np.asarray(Wq[:, g * 512 + j * D: g * 512 + (j + 1) * D]))
                for j in range(HPG)
            ],
            axis=0,
        ).reshape(HPG, 128, ECH * D)
        wk = to_sb_layout(np.asarray(Wk[:, g * D:(g + 1) * D]))
        wv = to_sb_layout(np.asarray(Wv[:, g * D:(g + 1) * D]))
        wo = np.ascontiguousarray(Wo[g * 512:(g + 1) * 512, :]).astype(bf).reshape(
            HPG, 128, EMBED
        )
        bqc = np.ascontiguousarray(
            bq[g * 512:(g + 1) * 512].reshape(HPG, 128).T
        ).astype(np.float32)
        bkc = np.asarray(bk[g * D:(g + 1) * D]).reshape(128, 1).astype(np.float32)
        bvc = np.asarray(bv[g * D:(g + 1) * D]).reshape(1, 128).astype(bf)
        in_maps.append(
            {
                "xt": xt,
                "wq": wq,
                "wk": wk,
                "wv": wv,
                "wo": wo,
                "bq": bqc,
                "bk": bkc,
                "bv": bvc,
            }
        )
    return in_maps


def combine_outputs(results, bo):
    out = np.empty((2, T, EMBED), dtype=np.float32)
    for b in range(2):
        acc = results[b * 4]["out"].astype(np.float32)
        for g in range(1, 4):
            acc += results[b * 4 + g]["out"].astype(np.float32)
        out[b] = acc + np.asarray(bo)[None, :].astype(np.float32)
    return out


def kernel(x, Wq, bq, Wk, bk, Wv, bv, Wo, bo):
    from concourse.bass_utils import run_bass_kernel_spmd

    nc = build_program()
    in_maps = prepare_in_maps(x, Wq, bq, Wk, bk, Wv, bv, Wo, bo)
    res = run_bass_kernel_spmd(nc, in_maps, list(range(NCORES)))
    return combine_outputs(res.results, np.asarray(bo))


# revision 8
# speedup vs baseline: 1.0486x; 1.0072x over previous
"""Grouped-Query Attention on 8 Trainium2 NeuronCores — v2.

Sharding: TP-4 over KV groups x DP-2 over batch.
Core c handles batch b = c // 4, group g = c % 4 (4 query heads, 1 KV group).

Differences vs v1:
  - V is projected directly into natural [t, d] layout in phase 1
    (stationary = xt tile, moving = Wv chunk) -- no PE transposes.
  - Softmax denominators: E accumulated over tk on DVE (bf16 2x), then a
    single allones[128,128] @ accE matmul broadcasts the denominator --
    removes 160 sum-matmuls + 16 broadcast-matmuls from PE.
  - Causal masking via Pool-engine affine_select (in-place on E).
  - Software-pipelined PE emission: S(h) pairs interleave with PV(h-1)
    pairs and O-projection(qc-1) filler matmuls.
  - DMA: wq split per head and interleaved with the xt stream; weights
    pre-transposed on host for 4KB-contiguous descriptors.
"""

import numpy as np
import ml_dtypes

EMBED = 2048
T = 2048
D = 128           # head dim
NQH = 16          # query heads
NG = 4            # kv groups
HPG = NQH // NG   # query heads per group = 4
NCORES = 8
ECH = EMBED // 128   # 16 contraction chunks
TC = T // 512        # 4 t-chunks of 512
TT = T // 128        # 16 t-tiles of 128
SCALE = 1.0 / float(np.sqrt(D))

_PROG = {}


def build_program():
    if "nc" in _PROG:
        return _PROG["nc"]

    from contextlib import ExitStack
    import concourse.mybir as mybir
    from concourse import bacc, tile

    # Drop redundant consecutive Ldweights with identical keys (the Tile
    # legalizer emits one per Matmult even when the stationary is unchanged).
    if not getattr(tile.tile_legalize, "_ldw_dedup", False):
        _orig_legalize = tile.tile_legalize

        def _dedup_legalize(ordered, nc_):
            ordered = _orig_legalize(ordered, nc_)
            for bb, insts in ordered.items():
                out = []
                state = None
                for inst in insts:
                    tn = type(inst).__name__
                    if tn == "InstLdweights":
                        key = (
                            str(inst.ins[0]),
                            str(getattr(inst, "is_transpose", None)),
                            str(getattr(inst, "tile_position", None)),
                            str(getattr(inst, "perf_mode", None)),
                        )
                        if key == state:
                            continue
                        state = key
                    out.append(inst)
                ordered[bb] = out
            return ordered

        _dedup_legalize._ldw_dedup = True
        tile.tile_legalize = _dedup_legalize

    dt = mybir.dt
    BF = dt.bfloat16
    F32 = dt.float32
    AF = mybir.ActivationFunctionType

    nc = bacc.Bacc("TRN2", target_bir_lowering=False, debug=False)

    xt_d = nc.dram_tensor("xt", [ECH, 128, T], BF, kind="ExternalInput")
    wq_d = nc.dram_tensor("wq", [HPG, 128, ECH * D], BF, kind="ExternalInput")
    wk_d = nc.dram_tensor("wk", [128, ECH * D], BF, kind="ExternalInput")
    wv_d = nc.dram_tensor("wv", [128, ECH * D], BF, kind="ExternalInput")
    wo_d = nc.dram_tensor("wo", [HPG, 128, EMBED], BF, kind="ExternalInput")
    bq_d = nc.dram_tensor("bq", [128, HPG], F32, kind="ExternalInput")
    bk_d = nc.dram_tensor("bk", [128, 1], F32, kind="ExternalInput")
    bv_d = nc.dram_tensor("bv", [1, 128], BF, kind="ExternalInput")
    out_d = nc.dram_tensor("out", [T, EMBED], BF, kind="ExternalOutput")

    with tile.TileContext(nc) as tc, ExitStack() as ctx:
        pers = ctx.enter_context(tc.tile_pool(name="pers", bufs=1))

        wq_sb = pers.tile([128, HPG, ECH * D], BF)
        wk_sb = pers.tile([128, ECH * D], BF)
        wv_sb = pers.tile([128, ECH * D], BF)
        wo_sb = pers.tile([128, HPG, EMBED], BF)
        bq_sb = pers.tile([128, HPG], F32)
        bk_sb = pers.tile([128, 1], F32)
        bv_sb = pers.tile([1, 128], BF)
        qT_sb = pers.tile([128, HPG, T], BF)
        kT_sb = pers.tile([128, T], BF)
        v_sb = pers.tile([128, TT, D], BF)
        nT_a = pers.tile([128, HPG, 512], BF)
        nT_b = pers.tile([128, HPG, 512], BF)
        ones1 = pers.tile([1, 128], BF)
        allones = pers.tile([128, 128], BF)

        # ---- Phase 1: projections ----
        if True:
            xt_sb = pers.tile([128, ECH, T], BF)

            # All input DMAs on the sync queue in priority order (the DMA
            # engines are a shared serial resource): wk/wv + biases first,
            # then xt chunks with wq heads interleaved, wo last (needed only
            # by the O-projection ~60us later).
            nc.sync.dma_start(wk_sb[:, 0:D], wk_d[:, 0:D])
            nc.sync.dma_start(xt_sb[:, 0, 0:1024], xt_d[0][:, 0:1024])
            nc.sync.dma_start(xt_sb[:, 0, 1024:], xt_d[0][:, 1024:])
            nc.sync.dma_start(wk_sb[:, D:2 * D], wk_d[:, D:2 * D])
            nc.sync.dma_start(xt_sb[:, 1, :], xt_d[1])
            nc.sync.dma_start(wv_sb[:, 0:D], wv_d[:, 0:D])
            nc.sync.dma_start(wk_sb[:, 2 * D:], wk_d[:, 2 * D:])
            nc.sync.dma_start(xt_sb[:, 2, :], xt_d[2])
            nc.sync.dma_start(wv_sb[:, D:], wv_d[:, D:])
            nc.sync.dma_start(bk_sb[:], bk_d[:])
            nc.sync.dma_start(bv_sb[:], bv_d[:])
            nc.sync.dma_start(bq_sb[:], bq_d[:])
            wq_next = 0
            for ec in range(3, ECH):
                nc.sync.dma_start(xt_sb[:, ec, :], xt_d[ec])
                if ec % 6 == 2 and wq_next < HPG:
                    nc.sync.dma_start(wq_sb[:, wq_next, :], wq_d[wq_next])
                    wq_next += 1
            while wq_next < HPG:
                nc.sync.dma_start(wq_sb[:, wq_next, :], wq_d[wq_next])
                wq_next += 1
            nc.sync.dma_start(wo_sb[:], wo_d.ap().rearrange("h p e -> p h e"))

            nc.gpsimd.memset(ones1[:], 1.0)
            nc.gpsimd.memset(allones[:], 1.0)

            # --- phase-1 PSUM: v accumulates on the right stack, kT on
            # the left.  kT's pool releases into ring1 (q half-passes + s2
            # pairs); v's pool releases into the po/pf pools.  No pool
            # barrier separates phase 1 from attention.
            kvB = tc.alloc_tile_pool(name="kvB", bufs=1, space="PSUM",
                                     side="right")
            vps = kvB.tile([128, TT, D], F32, tag="v", name="vps")
            kvA = tc.alloc_tile_pool(name="kvA", bufs=1, space="PSUM",
                                     side="left")
            kps = kvA.tile([128, T], F32, tag="k", name="kps")
            def v_chunk(ec):
                wvc = wv_sb[:, ec * D:(ec + 1) * D]
                for tt in range(TT):
                    # 4 tt-tiles share a PSUM bank; `start` zeroes the whole
                    # 2KB bank region, so only the bank's first write starts
                    # the group and its last write stops it.
                    nc.tensor.matmul(
                        vps[:, tt, :],
                        xt_sb[:, ec, tt * D:(tt + 1) * D],
                        wvc,
                        start=(ec == 0 and tt % 4 == 0),
                        stop=False,
                    )

            # v lags k by one chunk so the early PE work needs a thinner
            # DMA prefix (wv can land after the first two xt chunks)
            for ec in range(ECH):
                wkc = wk_sb[:, ec * D:(ec + 1) * D]
                for t5 in range(TC):
                    nc.tensor.matmul(
                        kps[:, t5 * 512:(t5 + 1) * 512],
                        wkc,
                        xt_sb[:, ec, t5 * 512:(t5 + 1) * 512],
                        start=(ec == 0),
                        stop=(ec == ECH - 1),
                    )
                if ec >= 1:
                    v_chunk(ec - 1)
            v_chunk(ECH - 1)
            # fold bv in as a rank-1 accumulation, closing each group
            for tt in range(TT):
                nc.tensor.matmul(
                    vps[:, tt, :], ones1[:], bv_sb[:],
                    start=False, stop=(tt % 4 == 3),
                )
            # drains: kT spread over ACT/DVE/Pool; v split DVE/Pool
            nc.scalar.activation(
                kT_sb[:], kps[:], AF.Identity, bias=bk_sb[:]
            )
            nc.vector.tensor_copy(v_sb[:], vps[:])
            kvA.release()
            ring1 = tc.alloc_tile_pool(name="ring1", bufs=2, space="PSUM",
                                       side="left")
            kvB.release()
            psoP = tc.alloc_tile_pool(name="psoP", bufs=2, space="PSUM",
                                      side="right")
            psfP = tc.alloc_tile_pool(name="psfP", bufs=2, space="PSUM",
                                      side="right")

            def q_half(j, th):
                ps = ring1.tile([128, 1024], F32, tag="r", name=f"q{j}h{th}")
                for ec in range(ECH):
                    lhsT = wq_sb[:, j, ec * D:(ec + 1) * D]
                    for t5 in range(2):
                        lo = th * 1024 + t5 * 512
                        nc.tensor.matmul(
                            ps[:, t5 * 512:(t5 + 1) * 512],
                            lhsT,
                            xt_sb[:, ec, lo:lo + 512],
                            start=(ec == 0),
                            stop=(ec == ECH - 1),
                        )
                if th == 1:
                    nc.scalar.activation(
                        qT_sb[:, j, th * 1024:(th + 1) * 1024], ps[:],
                        AF.Identity, bias=bq_sb[:, j:j + 1],
                    )
                else:
                    nc.vector.tensor_scalar_add(
                        qT_sb[:, j, th * 1024:(th + 1) * 1024], ps[:],
                        bq_sb[:, j:j + 1],
                    )

        # ---- Phase 2: attention + O-projection, software-pipelined ----
        with (
            tc.tile_pool(name="eb", bufs=2) as ebp,
            tc.tile_pool(name="acp", bufs=2) as acp,
            tc.tile_pool(name="rcp", bufs=2) as rcp,
            tc.tile_pool(name="fsb", bufs=4) as fsb,
        ):
            nT_for = {}

            def dg_off(qc, tk):
                # columns [0, off) of tile tk are fully below the causal
                # diagonal (masked out) -- skip computing them entirely
                return max(0, (tk - 4 * qc)) * D if tk >= 4 * qc else 0

            def s_pair(h, qc, tkp, nk, E, accE):
                """Two S matmuls -> exp pair -> mask diag -> accumulate."""
                s2 = ring1.tile([128, 2, 512], F32, tag="r", name="s2t")
                # both tiles of a pair computed at the pair's min diagonal
                # offset so the exp reads a fully-written region
                woff = dg_off(qc, tkp * 2)
                for u in range(2):
                    tk = tkp * 2 + u
                    nc.tensor.matmul(
                        s2[:, u, woff:],
                        kT_sb[:, tk * D:(tk + 1) * D],
                        qT_sb[:, h, qc * 512 + woff:(qc + 1) * 512],
                        start=True,
                        stop=True,
                    )
                nc.scalar.activation(
                    E[:, tkp * 2:tkp * 2 + 2, woff:], s2[:, :, woff:],
                    AF.Exp, scale=SCALE
                )
                for u in range(2):
                    tk = tkp * 2 + u
                    off = dg_off(qc, tk)
                    if tk >= 4 * qc:
                        # zero E[p, woff+f] where woff + f < off + p
                        nc.gpsimd.affine_select(
                            out=E[:, tk, woff:],
                            in_=E[:, tk, woff:],
                            pattern=[[1, 512 - woff]],
                            compare_op=mybir.AluOpType.is_ge,
                            fill=0.0,
                            base=woff - off,
                            channel_multiplier=-1,
                        )
                    if tk == 0:
                        nc.vector.tensor_copy(accE[:], E[:, 0, :])
                    else:
                        nc.vector.tensor_add(
                            accE[:, woff:], accE[:, woff:], E[:, tk, woff:]
                        )

            def pv_pair(h, qc, tkp, E, po, nk):
                for u in range(2):
                    tk = tkp * 2 + u
                    off = dg_off(qc, tk)
                    nc.tensor.matmul(
                        po[:, off:], v_sb[:, tk, :], E[:, tk, off:],
                        start=(tk == 0), stop=(tk == nk - 1),
                    )

            def sums_b(h, qc, accE, po, nT):
                sumsB = psfP.tile([128, 512], F32, tag="pf", name="sumsBt")
                nc.tensor.matmul(
                    sumsB[:], allones[:], accE[:], start=True, stop=True
                )
                recipS = rcp.tile([128, 512], F32, tag="recip", name="recipSt")
                nc.vector.reciprocal(recipS[:], sumsB[:])
                nc.vector.tensor_mul(nT[:, h, :], po[:], recipS[:])

            def o_tile(qc_prev, idx, drain_eng, split_drain=False,
                       alt_q=False):
                """One O-projection output tile: 4 matmuls + copy + DMA."""
                qt, ecol = divmod(idx, 4)
                nT = nT_for[qc_prev]
                pf = psfP.tile([128, 512], F32, tag="pf", name="pft")
                for h in range(HPG):
                    nc.tensor.matmul(
                        pf[:],
                        nT[:, h, qt * 128:(qt + 1) * 128],
                        wo_sb[:, h, ecol * 512:(ecol + 1) * 512],
                        start=(h == 0),
                        stop=(h == HPG - 1),
                    )
                f_t = fsb.tile([128, 512], BF, tag="f", name="ftt")
                row = qc_prev * 4 + qt
                if split_drain:
                    # end-of-program: halve latency by draining on both
                    # engines and overlapping the two half DMAs
                    nc.scalar.activation(f_t[:, :256], pf[:, :256], AF.Identity)
                    nc.vector.tensor_copy(f_t[:, 256:], pf[:, 256:])
                    nc.sync.dma_start(
                        out_d[row * 128:(row + 1) * 128,
                              ecol * 512:ecol * 512 + 256],
                        f_t[:, :256],
                    )
                    nc.sync.dma_start(
                        out_d[row * 128:(row + 1) * 128,
                              ecol * 512 + 256:(ecol + 1) * 512],
                        f_t[:, 256:],
                    )
                    return
                if drain_eng == 0:
                    nc.scalar.activation(f_t[:], pf[:], AF.Identity)
                else:
                    nc.vector.tensor_copy(f_t[:], pf[:])
                dq = nc.scalar if alt_q else nc.sync
                dq.dma_start(
                    out_d[row * 128:(row + 1) * 128,
                          ecol * 512:(ecol + 1) * 512],
                    f_t[:],
                )

            # Global depth-2 pipeline over tasks (qc, h): S(task i) pairs
            # interleave with PV(task i-2) pairs, with O(qc-1) filler tiles
            # inserted by a deficit model (ACT exp needs ~1140ns per pair vs
            # ~426ns of PE work per S or PV pair; an O tile is ~852ns).
            tasks = [(qc, h) for qc in range(TC) for h in range(HPG)]
            NT = len(tasks)
            npair_of = lambda i: 2 * (tasks[i][0] + 1)
            nT_for[0] = nT_a
            nT_for[1] = nT_b
            nT_for[2] = nT_a
            nT_for[3] = nT_b

            Es, accEs, pos = {}, {}, {}

            def get_e(i):
                if i not in Es:
                    Es[i] = ebp.tile([128, 16, 512], BF, tag="E", name=f"E{i}")
                    accEs[i] = acp.tile([128, 512], BF, tag="acc",
                                        name=f"acc{i}")
                return Es[i], accEs[i]

            def get_po(i):
                if i not in pos:
                    pos[i] = psoP.tile([128, 512], F32, tag="po", name=f"po{i}")
                return pos[i]

            state = {"deficit": 0.0, "drain": 0, "fillers": [], "fi": 0,
                     "fqc": None}

            def fill_one():
                if state["fi"] < len(state["fillers"]):
                    # during the last q-chunk the exp stream saturates ACT;
                    # keep its filler drains off that engine
                    de = 1 if state["fqc"] == TC - 2 else state["drain"]
                    o_tile(state["fqc"], state["fillers"][state["fi"]], de)
                    state["drain"] ^= 1
                    state["fi"] += 1
                    state["deficit"] -= 852.0
                    return True
                return False

            def drain_deficit(slack):
                while state["deficit"] > slack and fill_one():
                    pass

            def emit_s(i, p):
                qc, h = tasks[i]
                E, accE = get_e(i)
                s_pair(h, qc, p, 4 * (qc + 1), E, accE)
                state["deficit"] += 1340.0 - 426.0

            def emit_pv(i, p):
                qc, h = tasks[i]
                pv_pair(h, qc, p, Es[i], get_po(i), 4 * (qc + 1))
                state["deficit"] -= 426.0

            def emit_sums(i):
                qc, h = tasks[i]
                sums_b(h, qc, accEs[i], pos[i], nT_for[qc])

            # interleave q half-passes with qc0 attention tasks so the
            # exp latency of the smallest q-chunk hides under projections
            q_half(0, 0)
            q_half(0, 1)
            q_half(1, 0)
            q_half(1, 1)
            for p in range(npair_of(0)):
                emit_s(0, p)
            q_half(2, 0)
            for p in range(npair_of(1)):
                emit_s(1, p)
                emit_pv(0, p)
            emit_sums(0)
            q_half(2, 1)
            q_half(3, 0)
            for p in range(npair_of(2)):
                emit_s(2, p)
                emit_pv(1, p)
            emit_sums(1)
            q_half(3, 1)
            state["deficit"] = 0.0

            for i in range(3, NT + 1):
                if i < NT:
                    qc, h = tasks[i]
                    if h == 1 and qc > 0:
                        # O(qc-1) fillers become legal here: sums(qc-1, h3)
                        # was emitted at the end of the previous task, so the
                        # whole nT(qc-1) buffer has its writers queued.  Old
                        # leftovers must flush first (their nT buffer gets
                        # overwritten by sums(qc, h0) at the end of this
                        # task).
                        while fill_one():
                            pass
                        state["fillers"] = list(range(16))
                        state["fi"] = 0
                        state["fqc"] = qc - 1
                    for p in range(npair_of(i)):
                        emit_s(i, p)
                        if i >= 1 and p < npair_of(i - 1):
                            emit_pv(i - 1, p)
                        drain_deficit(852.0)
                else:
                    for p in range(npair_of(i - 1)):
                        emit_pv(i - 1, p)
                        drain_deficit(0.0)
                if i >= 1:
                    emit_sums(i - 1)

            # tail: O-projection for the last q-chunk
            while fill_one():
                pass
            for idx in range(12):
                o_tile(TC - 1, idx, idx & 1)
            for idx in range(12, 16):
                o_tile(TC - 1, idx, idx & 1, alt_q=(idx & 1 == 0))

            psfP.release()
            psoP.release()
            ring1.release()

    nc.compile()
    _PROG["nc"] = nc
    return nc


def prepare_in_maps(x, Wq, bq, Wk, bk, Wv, bv, Wo, bo):
    bf = ml_dtypes.bfloat16

    def to_sb_layout(W):  # [E, cols] -> [128, ECH*cols] partition-major
        cols = W.shape[1]
        return np.ascontiguousarray(
            W.reshape(ECH, 128, cols).transpose(1, 0, 2).reshape(128, ECH * cols)
        ).astype(bf)

    in_maps = []
    for c in range(NCORES):
        b, g = c // 4, c % 4
        xt = np.ascontiguousarray(np.asarray(x[b]).T).astype(bf).reshape(
            ECH, 128, T
        )
        wq = np.stack(
            [
                to_sb_layout(np.asarray(Wq[:, g * 512 + j * D: g * 512 + (j + 1) * D]))
                for j in range(HPG)
            ],
            axis=0,
        ).reshape(HPG, 128, ECH * D)
        wk = to_sb_layout(np.asarray(Wk[:, g * D:(g + 1) * D]))
        wv = to_sb_layout(np.asarray(Wv[:, g * D:(g + 1) * D]))
        wo = np.ascontiguousarray(Wo[g * 512:(g + 1) * 512, :]).astype(bf).reshape(
            HPG, 128, EMBED
        )
        bqc = np.ascontiguousarray(
            bq[g * 512:(g + 1) * 512].reshape(HPG, 128).T
        ).astype(np.float32)
        bkc = np.asarray(bk[g * D:(g + 1) * D]).reshape(128, 1).astype(np.float32)
        bvc = np.asarray(bv[g * D:(g + 1) * D]).reshape(1, 128).astype(bf)
        in_maps.append(
            {
                "xt": xt,
                "wq": wq,
                "wk": wk,
                "wv": wv,
                "wo": wo,
                "bq": bqc,
                "bk": bkc,
                "bv": bvc,
            }
        )
    return in_maps


def combine_outputs(results, bo):
    out = np.empty((2, T, EMBED), dtype=np.float32)
    for b in range(2):
        acc = results[b * 4]["out"].astype(np.float32)
        for g in range(1, 4):
            acc += results[b * 4 + g]["out"].astype(np.float32)
        out[b] = acc + np.asarray(bo)[None, :].astype(np.float32)
    return out


def kernel(x, Wq, bq, Wk, bk, Wv, bv, Wo, bo):
    from concourse.bass_utils import run_bass_kernel_spmd

    nc = build_program()
    in_maps = prepare_in_maps(x, Wq, bq, Wk, bk, Wv, bv, Wo, bo)
    res = run_bass_kernel_spmd(nc, in_maps, list(range(NCORES)))
    return combine_outputs(res.results, np.asarray(bo))


# revision 9
# speedup vs baseline: 1.0515x; 1.0028x over previous
"""Grouped-Query Attention on 8 Trainium2 NeuronCores — v2.

Sharding: TP-4 over KV groups x DP-2 over batch.
Core c handles batch b = c // 4, group g = c % 4 (4 query heads, 1 KV group).

Differences vs v1:
  - V is projected directly into natural [t, d] layout in phase 1
    (stationary = xt tile, moving = Wv chunk) -- no PE transposes.
  - Softmax denominators: E accumulated over tk on DVE (bf16 2x), then a
    single allones[128,128] @ accE matmul broadcasts the denominator --
    removes 160 sum-matmuls + 16 broadcast-matmuls from PE.
  - Causal masking via Pool-engine affine_select (in-place on E).
  - Software-pipelined PE emission: S(h) pairs interleave with PV(h-1)
    pairs and O-projection(qc-1) filler matmuls.
  - DMA: wq split per head and interleaved with the xt stream; weights
    pre-transposed on host for 4KB-contiguous descriptors.
"""

import numpy as np
import ml_dtypes

EMBED = 2048
T = 2048
D = 128           # head dim
NQH = 16          # query heads
NG = 4            # kv groups
HPG = NQH // NG   # query heads per group = 4
NCORES = 8
ECH = EMBED // 128   # 16 contraction chunks
TC = T // 512        # 4 t-chunks of 512
TT = T // 128        # 16 t-tiles of 128
SCALE = 1.0 / float(np.sqrt(D))

_PROG = {}


def build_program():
    if "nc" in _PROG:
        return _PROG["nc"]

    from contextlib import ExitStack
    import concourse.mybir as mybir
    from concourse import bacc, tile

    # Drop redundant consecutive Ldweights with identical keys (the Tile
    # legalizer emits one per Matmult even when the stationary is unchanged).
    if not getattr(tile.tile_legalize, "_ldw_dedup", False):
        _orig_legalize = tile.tile_legalize

        def _dedup_legalize(ordered, nc_):
            ordered = _orig_legalize(ordered, nc_)
            for bb, insts in ordered.items():
                out = []
                state = None
                for inst in insts:
                    tn = type(inst).__name__
                    if tn == "InstLdweights":
                        key = (
                            str(inst.ins[0]),
                            str(getattr(inst, "is_transpose", None)),
                            str(getattr(inst, "tile_position", None)),
                            str(getattr(inst, "perf_mode", None)),
                        )
                        if key == state:
                            continue
                        state = key
                    out.append(inst)
                ordered[bb] = out
            return ordered

        _dedup_legalize._ldw_dedup = True
        tile.tile_legalize = _dedup_legalize

    dt = mybir.dt
    BF = dt.bfloat16
    F32 = dt.float32
    AF = mybir.ActivationFunctionType

    nc = bacc.Bacc("TRN2", target_bir_lowering=False, debug=False)

    xt_d = nc.dram_tensor("xt", [ECH, 128, T], BF, kind="ExternalInput")
    wq_d = nc.dram_tensor("wq", [HPG, 128, ECH * D], BF, kind="ExternalInput")
    wk_d = nc.dram_tensor("wk", [128, ECH * D], BF, kind="ExternalInput")
    wv_d = nc.dram_tensor("wv", [128, ECH * D], BF, kind="ExternalInput")
    wo_d = nc.dram_tensor("wo", [HPG, 128, EMBED], BF, kind="ExternalInput")
    bq_d = nc.dram_tensor("bq", [128, HPG], F32, kind="ExternalInput")
    bk_d = nc.dram_tensor("bk", [128, 1], F32, kind="ExternalInput")
    bv_d = nc.dram_tensor("bv", [1, 128], BF, kind="ExternalInput")
    out_d = nc.dram_tensor("out", [T, EMBED], BF, kind="ExternalOutput")

    with tile.TileContext(nc) as tc, ExitStack() as ctx:
        pers = ctx.enter_context(tc.tile_pool(name="pers", bufs=1))

        wq_sb = pers.tile([128, HPG, ECH * D], BF)
        wk_sb = pers.tile([128, ECH * D], BF)
        wv_sb = pers.tile([128, ECH * D], BF)
        wo_sb = pers.tile([128, HPG, EMBED], BF)
        bq_sb = pers.tile([128, HPG], F32)
        bk_sb = pers.tile([128, 1], F32)
        bv_sb = pers.tile([1, 128], BF)
        qT_sb = pers.tile([128, HPG, T], BF)
        kT_sb = pers.tile([128, T], BF)
        v_sb = pers.tile([128, TT, D], BF)
        nT_a = pers.tile([128, HPG, 512], BF)
        nT_b = pers.tile([128, HPG, 512], BF)
        ones1 = pers.tile([1, 128], BF)
        allones = pers.tile([128, 128], BF)

        # ---- Phase 1: projections ----
        if True:
            xt_sb = pers.tile([128, ECH, T], BF)

            # All input DMAs on the sync queue in priority order (the DMA
            # engines are a shared serial resource): wk/wv + biases first,
            # then xt chunks with wq heads interleaved, wo last (needed only
            # by the O-projection ~60us later).
            nc.sync.dma_start(wk_sb[:, 0:D], wk_d[:, 0:D])
            nc.sync.dma_start(xt_sb[:, 0, 0:1024], xt_d[0][:, 0:1024])
            nc.sync.dma_start(xt_sb[:, 0, 1024:], xt_d[0][:, 1024:])
            nc.sync.dma_start(wk_sb[:, D:2 * D], wk_d[:, D:2 * D])
            nc.sync.dma_start(xt_sb[:, 1, :], xt_d[1])
            nc.sync.dma_start(wv_sb[:, 0:D], wv_d[:, 0:D])
            nc.sync.dma_start(wk_sb[:, 2 * D:], wk_d[:, 2 * D:])
            nc.sync.dma_start(xt_sb[:, 2, :], xt_d[2])
            nc.sync.dma_start(wv_sb[:, D:], wv_d[:, D:])
            nc.sync.dma_start(bk_sb[:], bk_d[:])
            nc.sync.dma_start(bv_sb[:], bv_d[:])
            nc.sync.dma_start(bq_sb[:], bq_d[:])
            wq_next = 0
            for ec in range(3, ECH):
                nc.sync.dma_start(xt_sb[:, ec, :], xt_d[ec])
                if ec % 6 == 2 and wq_next < HPG:
                    nc.sync.dma_start(wq_sb[:, wq_next, :], wq_d[wq_next])
                    wq_next += 1
            while wq_next < HPG:
                nc.sync.dma_start(wq_sb[:, wq_next, :], wq_d[wq_next])
                wq_next += 1
            nc.sync.dma_start(wo_sb[:], wo_d.ap().rearrange("h p e -> p h e"))

            nc.gpsimd.memset(ones1[:], 1.0)
            nc.gpsimd.memset(allones[:], 1.0)

            # --- phase-1 PSUM: v accumulates on the right stack, kT on
            # the left.  kT's pool releases into ring1 (q half-passes + s2
            # pairs); v's pool releases into the po/pf pools.  No pool
            # barrier separates phase 1 from attention.
            kvB = tc.alloc_tile_pool(name="kvB", bufs=1, space="PSUM",
                                     side="right")
            vps = kvB.tile([128, TT, D], F32, tag="v", name="vps")
            kvA = tc.alloc_tile_pool(name="kvA", bufs=1, space="PSUM",
                                     side="left")
            kps = kvA.tile([128, T], F32, tag="k", name="kps")
            def v_chunk(ec):
                wvc = wv_sb[:, ec * D:(ec + 1) * D]
                for tt in range(TT):
                    # 4 tt-tiles share a PSUM bank; `start` zeroes the whole
                    # 2KB bank region, so only the bank's first write starts
                    # the group and its last write stops it.
                    nc.tensor.matmul(
                        vps[:, tt, :],
                        xt_sb[:, ec, tt * D:(tt + 1) * D],
                        wvc,
                        start=(ec == 0 and tt % 4 == 0),
                        stop=False,
                    )

            # v lags k by one chunk so the early PE work needs a thinner
            # DMA prefix (wv can land after the first two xt chunks)
            for ec in range(ECH):
                wkc = wk_sb[:, ec * D:(ec + 1) * D]
                for t5 in range(TC):
                    nc.tensor.matmul(
                        kps[:, t5 * 512:(t5 + 1) * 512],
                        wkc,
                        xt_sb[:, ec, t5 * 512:(t5 + 1) * 512],
                        start=(ec == 0),
                        stop=(ec == ECH - 1),
                    )
                if ec >= 1:
                    v_chunk(ec - 1)
            v_chunk(ECH - 1)
            # fold bv in as a rank-1 accumulation, closing each group
            for tt in range(TT):
                nc.tensor.matmul(
                    vps[:, tt, :], ones1[:], bv_sb[:],
                    start=False, stop=(tt % 4 == 3),
                )
            # drains: kT spread over ACT/DVE/Pool; v split DVE/Pool
            nc.scalar.activation(
                kT_sb[:], kps[:], AF.Identity, bias=bk_sb[:]
            )
            nc.vector.tensor_copy(v_sb[:], vps[:])
            kvA.release()
            ring1 = tc.alloc_tile_pool(name="ring1", bufs=2, space="PSUM",
                                       side="left")
            kvB.release()
            psoP = tc.alloc_tile_pool(name="psoP", bufs=2, space="PSUM",
                                      side="right")
            psfP = tc.alloc_tile_pool(name="psfP", bufs=2, space="PSUM",
                                      side="right")

            def q_half(j, th):
                ps = ring1.tile([128, 1024], F32, tag="r", name=f"q{j}h{th}")
                for ec in range(ECH):
                    lhsT = wq_sb[:, j, ec * D:(ec + 1) * D]
                    for t5 in range(2):
                        lo = th * 1024 + t5 * 512
                        nc.tensor.matmul(
                            ps[:, t5 * 512:(t5 + 1) * 512],
                            lhsT,
                            xt_sb[:, ec, lo:lo + 512],
                            start=(ec == 0),
                            stop=(ec == ECH - 1),
                        )
                if th == 1:
                    nc.scalar.activation(
                        qT_sb[:, j, th * 1024:(th + 1) * 1024], ps[:],
                        AF.Identity, bias=bq_sb[:, j:j + 1],
                    )
                else:
                    nc.vector.tensor_scalar_add(
                        qT_sb[:, j, th * 1024:(th + 1) * 1024], ps[:],
                        bq_sb[:, j:j + 1],
                    )

        # ---- Phase 2: attention + O-projection, software-pipelined ----
        with (
            tc.tile_pool(name="eb", bufs=2) as ebp,
            tc.tile_pool(name="acp", bufs=2) as acp,
            tc.tile_pool(name="rcp", bufs=2) as rcp,
            tc.tile_pool(name="fsb", bufs=4) as fsb,
        ):
            nT_for = {}

            def dg_off(qc, tk):
                # columns [0, off) of tile tk are fully below the causal
                # diagonal (masked out) -- skip computing them entirely
                return max(0, (tk - 4 * qc)) * D if tk >= 4 * qc else 0

            def s_pair(h, qc, tkp, nk, E, accE):
                """Two S matmuls -> exp pair -> mask diag -> accumulate."""
                s2 = ring1.tile([128, 2, 512], F32, tag="r", name="s2t")
                # both tiles of a pair computed at the pair's min diagonal
                # offset so the exp reads a fully-written region
                woff = dg_off(qc, tkp * 2)
                for u in range(2):
                    tk = tkp * 2 + u
                    nc.tensor.matmul(
                        s2[:, u, woff:],
                        kT_sb[:, tk * D:(tk + 1) * D],
                        qT_sb[:, h, qc * 512 + woff:(qc + 1) * 512],
                        start=True,
                        stop=True,
                    )
                nc.scalar.activation(
                    E[:, tkp * 2:tkp * 2 + 2, woff:], s2[:, :, woff:],
                    AF.Exp, scale=SCALE
                )
                for u in range(2):
                    tk = tkp * 2 + u
                    off = dg_off(qc, tk)
                    if tk >= 4 * qc:
                        # zero E[p, woff+f] where woff + f < off + p
                        nc.gpsimd.affine_select(
                            out=E[:, tk, woff:],
                            in_=E[:, tk, woff:],
                            pattern=[[1, 512 - woff]],
                            compare_op=mybir.AluOpType.is_ge,
                            fill=0.0,
                            base=woff - off,
                            channel_multiplier=-1,
                        )
                    if tk == 0:
                        nc.vector.tensor_copy(accE[:], E[:, 0, :])
                    else:
                        nc.vector.tensor_add(
                            accE[:, woff:], accE[:, woff:], E[:, tk, woff:]
                        )

            def pv_pair(h, qc, tkp, E, po, nk):
                for u in range(2):
                    tk = tkp * 2 + u
                    off = dg_off(qc, tk)
                    nc.tensor.matmul(
                        po[:, off:], v_sb[:, tk, :], E[:, tk, off:],
                        start=(tk == 0), stop=(tk == nk - 1),
                    )

            def sums_b(h, qc, accE, po, nT):
                sumsB = psfP.tile([128, 512], F32, tag="pf", name="sumsBt")
                nc.tensor.matmul(
                    sumsB[:], allones[:], accE[:], start=True, stop=True
                )
                recipS = rcp.tile([128, 512], F32, tag="recip", name="recipSt")
                nc.vector.reciprocal(recipS[:], sumsB[:])
                nc.vector.tensor_mul(nT[:, h, :], po[:], recipS[:])

            def o_tile(qc_prev, idx, drain_eng, split_drain=False,
                       alt_q=False):
                """One O-projection output tile: 4 matmuls + copy + DMA."""
                qt, ecol = divmod(idx, 4)
                nT = nT_for[qc_prev]
                pf = psfP.tile([128, 512], F32, tag="pf", name="pft")
                for h in range(HPG):
                    nc.tensor.matmul(
                        pf[:],
                        nT[:, h, qt * 128:(qt + 1) * 128],
                        wo_sb[:, h, ecol * 512:(ecol + 1) * 512],
                        start=(h == 0),
                        stop=(h == HPG - 1),
                    )
                f_t = fsb.tile([128, 512], BF, tag="f", name="ftt")
                row = qc_prev * 4 + qt
                if split_drain:
                    # end-of-program: halve latency by draining on both
                    # engines and overlapping the two half DMAs
                    nc.scalar.activation(f_t[:, :256], pf[:, :256], AF.Identity)
                    nc.vector.tensor_copy(f_t[:, 256:], pf[:, 256:])
                    nc.sync.dma_start(
                        out_d[row * 128:(row + 1) * 128,
                              ecol * 512:ecol * 512 + 256],
                        f_t[:, :256],
                    )
                    nc.sync.dma_start(
                        out_d[row * 128:(row + 1) * 128,
                              ecol * 512 + 256:(ecol + 1) * 512],
                        f_t[:, 256:],
                    )
                    return
                if drain_eng == 0:
                    nc.scalar.activation(f_t[:], pf[:], AF.Identity)
                else:
                    nc.vector.tensor_copy(f_t[:], pf[:])
                dq = nc.scalar if alt_q else nc.sync
                dq.dma_start(
                    out_d[row * 128:(row + 1) * 128,
                          ecol * 512:(ecol + 1) * 512],
                    f_t[:],
                )

            # Global depth-2 pipeline over tasks (qc, h): S(task i) pairs
            # interleave with PV(task i-2) pairs, with O(qc-1) filler tiles
            # inserted by a deficit model (ACT exp needs ~1140ns per pair vs
            # ~426ns of PE work per S or PV pair; an O tile is ~852ns).
            tasks = [(qc, h) for qc in range(TC) for h in range(HPG)]
            NT = len(tasks)
            npair_of = lambda i: 2 * (tasks[i][0] + 1)
            nT_for[0] = nT_a
            nT_for[1] = nT_b
            nT_for[2] = nT_a
            nT_for[3] = nT_b

            Es, accEs, pos = {}, {}, {}

            def get_e(i):
                if i not in Es:
                    Es[i] = ebp.tile([128, 16, 512], BF, tag="E", name=f"E{i}")
                    accEs[i] = acp.tile([128, 512], BF, tag="acc",
                                        name=f"acc{i}")
                return Es[i], accEs[i]

            def get_po(i):
                if i not in pos:
                    pos[i] = psoP.tile([128, 512], F32, tag="po", name=f"po{i}")
                return pos[i]

            state = {"deficit": 0.0, "drain": 0, "fillers": [], "fi": 0,
                     "fqc": None}

            def fill_one():
                if state["fi"] < len(state["fillers"]):
                    # during the last q-chunk the exp stream saturates ACT;
                    # keep its filler drains off that engine
                    de = 1 if state["fqc"] == TC - 2 else state["drain"]
                    o_tile(state["fqc"], state["fillers"][state["fi"]], de)
                    state["drain"] ^= 1
                    state["fi"] += 1
                    state["deficit"] -= 852.0
                    return True
                return False

            def drain_deficit(slack):
                while state["deficit"] > slack and fill_one():
                    pass

            def emit_s(i, p):
                qc, h = tasks[i]
                E, accE = get_e(i)
                s_pair(h, qc, p, 4 * (qc + 1), E, accE)
                state["deficit"] += 1340.0 - 426.0

            def emit_pv(i, p):
                qc, h = tasks[i]
                pv_pair(h, qc, p, Es[i], get_po(i), 4 * (qc + 1))
                state["deficit"] -= 426.0

            def emit_sums(i):
                qc, h = tasks[i]
                sums_b(h, qc, accEs[i], pos[i], nT_for[qc])

            # interleave q half-passes with qc0 attention tasks so the
            # exp latency of the smallest q-chunk hides under projections
            q_half(0, 0)
            q_half(0, 1)
            q_half(1, 0)
            for p in range(npair_of(0)):
                emit_s(0, p)
            q_half(1, 1)
            q_half(2, 0)
            for p in range(npair_of(1)):
                emit_s(1, p)
                emit_pv(0, p)
            q_half(2, 1)
            emit_sums(0)
            q_half(3, 0)
            for p in range(npair_of(2)):
                emit_s(2, p)
                emit_pv(1, p)
            q_half(3, 1)
            emit_sums(1)
            state["deficit"] = 0.0

            for i in range(3, NT + 1):
                if i < NT:
                    qc, h = tasks[i]
                    if h == 1 and qc > 0:
                        # O(qc-1) fillers become legal here: sums(qc-1, h3)
                        # was emitted at the end of the previous task, so the
                        # whole nT(qc-1) buffer has its writers queued.  Old
                        # leftovers must flush first (their nT buffer gets
                        # overwritten by sums(qc, h0) at the end of this
                        # task).
                        while fill_one():
                            pass
                        state["fillers"] = list(range(16))
                        state["fi"] = 0
                        state["fqc"] = qc - 1
                    for p in range(npair_of(i)):
                        emit_s(i, p)
                        if i >= 1 and p < npair_of(i - 1):
                            emit_pv(i - 1, p)
                        drain_deficit(852.0)
                else:
                    for p in range(npair_of(i - 1)):
                        emit_pv(i - 1, p)
                        drain_deficit(0.0)
                if i >= 1:
                    emit_sums(i - 1)

            # tail: O-projection for the last q-chunk
            while fill_one():
                pass
            for idx in range(12):
                o_tile(TC - 1, idx, idx & 1)
            for idx in range(12, 16):
                o_tile(TC - 1, idx, idx & 1, alt_q=(idx & 1 == 0))

            psfP.release()
            psoP.release()
            ring1.release()

    nc.compile()
    _PROG["nc"] = nc
    return nc


def prepare_in_maps(x, Wq, bq, Wk, bk, Wv, bv, Wo, bo):
    bf = ml_dtypes.bfloat16

    def to_sb_layout(W):  # [E, cols] -> [128, ECH*cols] partition-major
        cols = W.shape[1]
        return np.ascontiguousarray(
            W.reshape(ECH, 128, cols).transpose(1, 0, 2).reshape(128, ECH * cols)
        ).astype(bf)

    in_maps = []
    for c in range(NCORES):
        b, g = c // 4, c % 4
        xt = np.ascontiguousarray(np.asarray(x[b]).T).astype(bf).reshape(
            ECH, 128, T
        )
        wq = np.stack(
            [
                to_sb_layout(np.asarray(Wq[:, g * 512 + j * D: g * 512 + (j + 1) * D]))
                for j in range(HPG)
            ],
            axis=0,
        ).reshape(HPG, 128, ECH * D)
        wk = to_sb_layout(np.asarray(Wk[:, g * D:(g + 1) * D]))
        wv = to_sb_layout(np.asarray(Wv[:, g * D:(g + 1) * D]))
        wo = np.ascontiguousarray(Wo[g * 512:(g + 1) * 512, :]).astype(bf).reshape(
            HPG, 128, EMBED
        )
        bqc = np.ascontiguousarray(
            bq[g * 512:(g + 1) * 512].reshape(HPG, 128).T
        ).astype(np.float32)
        bkc = np.asarray(bk[g * D:(g + 1) * D]).reshape(128, 1).astype(np.float32)
        bvc = np.asarray(bv[g * D:(g + 1) * D]).reshape(1, 128).astype(bf)
        in_maps.append(
            {
                "xt": xt,
                "wq": wq,
                "wk": wk,
                "wv": wv,
                "wo": wo,
                "bq": bqc,
                "bk": bkc,
                "bv": bvc,
            }
        )
    return in_maps


def combine_outputs(results, bo):
    out = np.empty((2, T, EMBED), dtype=np.float32)
    for b in range(2):
        acc = results[b * 4]["out"].astype(np.float32)
        for g in range(1, 4):
            acc += results[b * 4 + g]["out"].astype(np.float32)
        out[b] = acc + np.asarray(bo)[None, :].astype(np.float32)
    return out


def kernel(x, Wq, bq, Wk, bk, Wv, bv, Wo, bo):
    from concourse.bass_utils import run_bass_kernel_spmd

    nc = build_program()
    in_maps = prepare_in_maps(x, Wq, bq, Wk, bk, Wv, bv, Wo, bo)
    res = run_bass_kernel_spmd(nc, in_maps, list(range(NCORES)))
    return combine_outputs(res.results, np.asarray(bo))


# revision 10
# speedup vs baseline: 1.0523x; 1.0008x over previous
"""Grouped-Query Attention on 8 Trainium2 NeuronCores — v2.

Sharding: TP-4 over KV groups x DP-2 over batch.
Core c handles batch b = c // 4, group g = c % 4 (4 query heads, 1 KV group).

Differences vs v1:
  - V is projected directly into natural [t, d] layout in phase 1
    (stationary = xt tile, moving = Wv chunk) -- no PE transposes.
  - Softmax denominators: E accumulated over tk on DVE (bf16 2x), then a
    single allones[128,128] @ accE matmul broadcasts the denominator --
    removes 160 sum-matmuls + 16 broadcast-matmuls from PE.
  - Causal masking via Pool-engine affine_select (in-place on E).
  - Software-pipelined PE emission: S(h) pairs interleave with PV(h-1)
    pairs and O-projection(qc-1) filler matmuls.
  - DMA: wq split per head and interleaved with the xt stream; weights
    pre-transposed on host for 4KB-contiguous descriptors.
"""

import numpy as np
import ml_dtypes

EMBED = 2048
T = 2048
D = 128           # head dim
NQH = 16          # query heads
NG = 4            # kv groups
HPG = NQH // NG   # query heads per group = 4
NCORES = 8
ECH = EMBED // 128   # 16 contraction chunks
TC = T // 512        # 4 t-chunks of 512
TT = T // 128        # 16 t-tiles of 128
SCALE = 1.0 / float(np.sqrt(D))

_PROG = {}


def build_program():
    if "nc" in _PROG:
        return _PROG["nc"]

    from contextlib import ExitStack
    import concourse.mybir as mybir
    from concourse import bacc, tile

    # Drop redundant consecutive Ldweights with identical keys (the Tile
    # legalizer emits one per Matmult even when the stationary is unchanged).
    if not getattr(tile.tile_legalize, "_ldw_dedup", False):
        _orig_legalize = tile.tile_legalize

        def _dedup_legalize(ordered, nc_):
            ordered = _orig_legalize(ordered, nc_)
            for bb, insts in ordered.items():
                out = []
                state = None
                for inst in insts:
                    tn = type(inst).__name__
                    if tn == "InstLdweights":
                        key = (
                            str(inst.ins[0]),
                            str(getattr(inst, "is_transpose", None)),
                            str(getattr(inst, "tile_position", None)),
                            str(getattr(inst, "perf_mode", None)),
                        )
                        if key == state:
                            continue
                        state = key
                    out.append(inst)
                ordered[bb] = out
            return ordered

        _dedup_legalize._ldw_dedup = True
        tile.tile_legalize = _dedup_legalize

    dt = mybir.dt
    BF = dt.bfloat16
    F32 = dt.float32
    AF = mybir.ActivationFunctionType

    nc = bacc.Bacc("TRN2", target_bir_lowering=False, debug=False)

    xt_d = nc.dram_tensor("xt", [ECH, 128, T], BF, kind="ExternalInput")
    wq_d = nc.dram_tensor("wq", [HPG, 128, ECH * D], BF, kind="ExternalInput")
    wk_d = nc.dram_tensor("wk", [128, ECH * D], BF, kind="ExternalInput")
    wv_d = nc.dram_tensor("wv", [128, ECH * D], BF, kind="ExternalInput")
    wo_d = nc.dram_tensor("wo", [HPG, 128, EMBED], BF, kind="ExternalInput")
    bq_d = nc.dram_tensor("bq", [128, HPG], F32, kind="ExternalInput")
    bk_d = nc.dram_tensor("bk", [128, 1], F32, kind="ExternalInput")
    bv_d = nc.dram_tensor("bv", [1, 128], BF, kind="ExternalInput")
    out_d = nc.dram_tensor("out", [T, EMBED], BF, kind="ExternalOutput")

    with tile.TileContext(nc) as tc, ExitStack() as ctx:
        pers = ctx.enter_context(tc.tile_pool(name="pers", bufs=1))

        wq_sb = pers.tile([128, HPG, ECH * D], BF)
        wk_sb = pers.tile([128, ECH * D], BF)
        wv_sb = pers.tile([128, ECH * D], BF)
        wo_sb = pers.tile([128, HPG, EMBED], BF)
        bq_sb = pers.tile([128, HPG], F32)
        bk_sb = pers.tile([128, 1], F32)
        bv_sb = pers.tile([1, 128], BF)
        qT_sb = pers.tile([128, HPG, T], BF)
        kT_sb = pers.tile([128, T], BF)
        v_sb = pers.tile([128, TT, D], BF)
        nT_a = pers.tile([128, HPG, 512], BF)
        nT_b = pers.tile([128, HPG, 512], BF)
        ones1 = pers.tile([1, 128], BF)
        allones = pers.tile([128, 128], BF)

        # ---- Phase 1: projections ----
        if True:
            xt_sb = pers.tile([128, ECH, T], BF)

            # All input DMAs on the sync queue in priority order (the DMA
            # engines are a shared serial resource): wk/wv + biases first,
            # then xt chunks with wq heads interleaved, wo last (needed only
            # by the O-projection ~60us later).
            nc.sync.dma_start(wk_sb[:, 0:D], wk_d[:, 0:D])
            nc.sync.dma_start(xt_sb[:, 0, 0:1024], xt_d[0][:, 0:1024])
            nc.sync.dma_start(xt_sb[:, 0, 1024:], xt_d[0][:, 1024:])
            nc.sync.dma_start(wk_sb[:, D:2 * D], wk_d[:, D:2 * D])
            nc.sync.dma_start(xt_sb[:, 1, :], xt_d[1])
            nc.sync.dma_start(wv_sb[:, 0:D], wv_d[:, 0:D])
            nc.sync.dma_start(wk_sb[:, 2 * D:], wk_d[:, 2 * D:])
            nc.sync.dma_start(xt_sb[:, 2, :], xt_d[2])
            nc.sync.dma_start(wv_sb[:, D:], wv_d[:, D:])
            nc.sync.dma_start(bk_sb[:], bk_d[:])
            nc.sync.dma_start(bv_sb[:], bv_d[:])
            nc.sync.dma_start(bq_sb[:], bq_d[:])
            wq_next = 0
            for ec in range(3, ECH):
                nc.sync.dma_start(xt_sb[:, ec, :], xt_d[ec])
                if ec % 6 == 2 and wq_next < HPG:
                    nc.sync.dma_start(wq_sb[:, wq_next, :], wq_d[wq_next])
                    wq_next += 1
            while wq_next < HPG:
                nc.sync.dma_start(wq_sb[:, wq_next, :], wq_d[wq_next])
                wq_next += 1
            nc.sync.dma_start(wo_sb[:], wo_d.ap().rearrange("h p e -> p h e"))

            nc.gpsimd.memset(ones1[:], 1.0)
            nc.gpsimd.memset(allones[:], 1.0)

            # --- phase-1 PSUM: v accumulates on the right stack, kT on
            # the left.  kT's pool releases into ring1 (q half-passes + s2
            # pairs); v's pool releases into the po/pf pools.  No pool
            # barrier separates phase 1 from attention.
            kvB = tc.alloc_tile_pool(name="kvB", bufs=1, space="PSUM",
                                     side="right")
            vps = kvB.tile([128, TT, D], F32, tag="v", name="vps")
            kvA = tc.alloc_tile_pool(name="kvA", bufs=1, space="PSUM",
                                     side="left")
            kps = kvA.tile([128, T], F32, tag="k", name="kps")
            def v_chunk(ec):
                wvc = wv_sb[:, ec * D:(ec + 1) * D]
                for tt in range(TT):
                    # 4 tt-tiles share a PSUM bank; `start` zeroes the whole
                    # 2KB bank region, so only the bank's first write starts
                    # the group and its last write stops it.
                    nc.tensor.matmul(
                        vps[:, tt, :],
                        xt_sb[:, ec, tt * D:(tt + 1) * D],
                        wvc,
                        start=(ec == 0 and tt % 4 == 0),
                        stop=False,
                    )

            # v lags k by one chunk so the early PE work needs a thinner
            # DMA prefix (wv can land after the first two xt chunks)
            for ec in range(ECH):
                wkc = wk_sb[:, ec * D:(ec + 1) * D]
                for t5 in range(TC):
                    nc.tensor.matmul(
                        kps[:, t5 * 512:(t5 + 1) * 512],
                        wkc,
                        xt_sb[:, ec, t5 * 512:(t5 + 1) * 512],
                        start=(ec == 0),
                        stop=(ec == ECH - 1),
                    )
                if ec >= 1:
                    v_chunk(ec - 1)
            v_chunk(ECH - 1)
            # fold bv in as a rank-1 accumulation, closing each group
            for tt in range(TT):
                nc.tensor.matmul(
                    vps[:, tt, :], ones1[:], bv_sb[:],
                    start=False, stop=(tt % 4 == 3),
                )
            # drains: kT spread over ACT/DVE/Pool; v split DVE/Pool
            nc.scalar.activation(
                kT_sb[:], kps[:], AF.Identity, bias=bk_sb[:]
            )
            nc.vector.tensor_copy(v_sb[:], vps[:])
            kvA.release()
            ring1 = tc.alloc_tile_pool(name="ring1", bufs=2, space="PSUM",
                                       side="left")
            kvB.release()
            psoP = tc.alloc_tile_pool(name="psoP", bufs=2, space="PSUM",
                                      side="right")
            psfP = tc.alloc_tile_pool(name="psfP", bufs=2, space="PSUM",
                                      side="right")

            def q_half(j, th):
                ps = ring1.tile([128, 1024], F32, tag="r", name=f"q{j}h{th}")
                for ec in range(ECH):
                    lhsT = wq_sb[:, j, ec * D:(ec + 1) * D]
                    for t5 in range(2):
                        lo = th * 1024 + t5 * 512
                        nc.tensor.matmul(
                            ps[:, t5 * 512:(t5 + 1) * 512],
                            lhsT,
                            xt_sb[:, ec, lo:lo + 512],
                            start=(ec == 0),
                            stop=(ec == ECH - 1),
                        )
                if th == 1:
                    nc.scalar.activation(
                        qT_sb[:, j, th * 1024:(th + 1) * 1024], ps[:],
                        AF.Identity, bias=bq_sb[:, j:j + 1],
                    )
                else:
                    nc.vector.tensor_scalar_add(
                        qT_sb[:, j, th * 1024:(th + 1) * 1024], ps[:],
                        bq_sb[:, j:j + 1],
                    )

        # ---- Phase 2: attention + O-projection, software-pipelined ----
        with (
            tc.tile_pool(name="eb", bufs=2) as ebp,
            tc.tile_pool(name="acp", bufs=2) as acp,
            tc.tile_pool(name="rcp", bufs=2) as rcp,
            tc.tile_pool(name="fsb", bufs=4) as fsb,
        ):
            nT_for = {}

            def dg_off(qc, tk):
                # columns [0, off) of tile tk are fully below the causal
                # diagonal (masked out) -- skip computing them entirely
                return max(0, (tk - 4 * qc)) * D if tk >= 4 * qc else 0

            def s_pair(h, qc, tkp, nk, E, accE):
                """Two S matmuls -> exp pair -> mask diag -> accumulate."""
                s2 = ring1.tile([128, 2, 512], F32, tag="r", name="s2t")
                # both tiles of a pair computed at the pair's min diagonal
                # offset so the exp reads a fully-written region
                woff = dg_off(qc, tkp * 2)
                for u in range(2):
                    tk = tkp * 2 + u
                    nc.tensor.matmul(
                        s2[:, u, woff:],
                        kT_sb[:, tk * D:(tk + 1) * D],
                        qT_sb[:, h, qc * 512 + woff:(qc + 1) * 512],
                        start=True,
                        stop=True,
                    )
                nc.scalar.activation(
                    E[:, tkp * 2:tkp * 2 + 2, woff:], s2[:, :, woff:],
                    AF.Exp, scale=SCALE
                )
                for u in range(2):
                    tk = tkp * 2 + u
                    off = dg_off(qc, tk)
                    if tk >= 4 * qc:
                        # zero E[p, woff+f] where woff + f < off + p
                        nc.gpsimd.affine_select(
                            out=E[:, tk, woff:],
                            in_=E[:, tk, woff:],
                            pattern=[[1, 512 - woff]],
                            compare_op=mybir.AluOpType.is_ge,
                            fill=0.0,
                            base=woff - off,
                            channel_multiplier=-1,
                        )
                    if tk == 0:
                        nc.vector.tensor_copy(accE[:], E[:, 0, :])
                    else:
                        nc.vector.tensor_add(
                            accE[:, off:], accE[:, off:], E[:, tk, off:]
                        )

            def pv_pair(h, qc, tkp, E, po, nk):
                for u in range(2):
                    tk = tkp * 2 + u
                    off = dg_off(qc, tk)
                    nc.tensor.matmul(
                        po[:, off:], v_sb[:, tk, :], E[:, tk, off:],
                        start=(tk == 0), stop=(tk == nk - 1),
                    )

            def sums_b(h, qc, accE, po, nT):
                sumsB = psfP.tile([128, 512], F32, tag="pf", name="sumsBt")
                nc.tensor.matmul(
                    sumsB[:], allones[:], accE[:], start=True, stop=True
                )
                recipS = rcp.tile([128, 512], F32, tag="recip", name="recipSt")
                nc.vector.reciprocal(recipS[:], sumsB[:])
                nc.vector.tensor_mul(nT[:, h, :], po[:], recipS[:])

            def o_tile(qc_prev, idx, drain_eng, split_drain=False,
                       alt_q=False):
                """One O-projection output tile: 4 matmuls + copy + DMA."""
                qt, ecol = divmod(idx, 4)
                nT = nT_for[qc_prev]
                pf = psfP.tile([128, 512], F32, tag="pf", name="pft")
                for h in range(HPG):
                    nc.tensor.matmul(
                        pf[:],
                        nT[:, h, qt * 128:(qt + 1) * 128],
                        wo_sb[:, h, ecol * 512:(ecol + 1) * 512],
                        start=(h == 0),
                        stop=(h == HPG - 1),
                    )
                f_t = fsb.tile([128, 512], BF, tag="f", name="ftt")
                row = qc_prev * 4 + qt
                if split_drain:
                    # end-of-program: halve latency by draining on both
                    # engines and overlapping the two half DMAs
                    nc.scalar.activation(f_t[:, :256], pf[:, :256], AF.Identity)
                    nc.vector.tensor_copy(f_t[:, 256:], pf[:, 256:])
                    nc.sync.dma_start(
                        out_d[row * 128:(row + 1) * 128,
                              ecol * 512:ecol * 512 + 256],
                        f_t[:, :256],
                    )
                    nc.sync.dma_start(
                        out_d[row * 128:(row + 1) * 128,
                              ecol * 512 + 256:(ecol + 1) * 512],
                        f_t[:, 256:],
                    )
                    return
                if drain_eng == 0:
                    nc.scalar.activation(f_t[:], pf[:], AF.Identity)
                else:
                    nc.vector.tensor_copy(f_t[:], pf[:])
                dq = nc.scalar if alt_q else nc.sync
                dq.dma_start(
                    out_d[row * 128:(row + 1) * 128,
                          ecol * 512:(ecol + 1) * 512],
                    f_t[:],
                )

            # Global depth-2 pipeline over tasks (qc, h): S(task i) pairs
            # interleave with PV(task i-2) pairs, with O(qc-1) filler tiles
            # inserted by a deficit model (ACT exp needs ~1140ns per pair vs
            # ~426ns of PE work per S or PV pair; an O tile is ~852ns).
            tasks = [(qc, h) for qc in range(TC) for h in range(HPG)]
            NT = len(tasks)
            npair_of = lambda i: 2 * (tasks[i][0] + 1)
            nT_for[0] = nT_a
            nT_for[1] = nT_b
            nT_for[2] = nT_a
            nT_for[3] = nT_b

            Es, accEs, pos = {}, {}, {}

            def get_e(i):
                if i not in Es:
                    Es[i] = ebp.tile([128, 16, 512], BF, tag="E", name=f"E{i}")
                    accEs[i] = acp.tile([128, 512], BF, tag="acc",
                                        name=f"acc{i}")
                return Es[i], accEs[i]

            def get_po(i):
                if i not in pos:
                    pos[i] = psoP.tile([128, 512], F32, tag="po", name=f"po{i}")
                return pos[i]

            state = {"deficit": 0.0, "drain": 0, "fillers": [], "fi": 0,
                     "fqc": None}

            def fill_one():
                if state["fi"] < len(state["fillers"]):
                    # during the last q-chunk the exp stream saturates ACT;
                    # keep its filler drains off that engine
                    de = 1 if state["fqc"] == TC - 2 else state["drain"]
                    o_tile(state["fqc"], state["fillers"][state["fi"]], de)
                    state["drain"] ^= 1
                    state["fi"] += 1
                    state["deficit"] -= 852.0
                    return True
                return False

            def drain_deficit(slack):
                while state["deficit"] > slack and fill_one():
                    pass

            def emit_s(i, p):
                qc, h = tasks[i]
                E, accE = get_e(i)
                s_pair(h, qc, p, 4 * (qc + 1), E, accE)
                state["deficit"] += 1340.0 - 426.0

            def emit_pv(i, p):
                qc, h = tasks[i]
                pv_pair(h, qc, p, Es[i], get_po(i), 4 * (qc + 1))
                state["deficit"] -= 426.0

            def emit_sums(i):
                qc, h = tasks[i]
                sums_b(h, qc, accEs[i], pos[i], nT_for[qc])

            # interleave q half-passes with qc0 attention tasks so the
            # exp latency of the smallest q-chunk hides under projections
            q_half(0, 0)
            q_half(0, 1)
            q_half(1, 0)
            for p in range(npair_of(0)):
                emit_s(0, p)
            q_half(1, 1)
            q_half(2, 0)
            for p in range(npair_of(1)):
                emit_s(1, p)
                emit_pv(0, p)
            q_half(2, 1)
            emit_sums(0)
            q_half(3, 0)
            for p in range(npair_of(2)):
                emit_s(2, p)
                emit_pv(1, p)
            q_half(3, 1)
            emit_sums(1)
            state["deficit"] = 0.0

            for i in range(3, NT + 1):
                if i < NT:
                    qc, h = tasks[i]
                    if h == 1 and qc > 0:
                        # O(qc-1) fillers become legal here: sums(qc-1, h3)
                        # was emitted at the end of the previous task, so the
                        # whole nT(qc-1) buffer has its writers queued.  Old
                        # leftovers must flush first (their nT buffer gets
                        # overwritten by sums(qc, h0) at the end of this
                        # task).
                        while fill_one():
                            pass
                        state["fillers"] = list(range(16))
                        state["fi"] = 0
                        state["fqc"] = qc - 1
                    for p in range(npair_of(i)):
                        emit_s(i, p)
                        if i >= 1 and p < npair_of(i - 1):
                            emit_pv(i - 1, p)
                        drain_deficit(852.0)
                else:
                    for p in range(npair_of(i - 1)):
                        emit_pv(i - 1, p)
                        drain_deficit(0.0)
                if i >= 1:
                    emit_sums(i - 1)

            # tail: O-projection for the last q-chunk
            while fill_one():
                pass
            for idx in range(12):
                o_tile(TC - 1, idx, idx & 1)
            for idx in range(12, 16):
                o_tile(TC - 1, idx, idx & 1, alt_q=(idx & 1 == 0))

            psfP.release()
            psoP.release()
            ring1.release()

    nc.compile()
    _PROG["nc"] = nc
    return nc


def prepare_in_maps(x, Wq, bq, Wk, bk, Wv, bv, Wo, bo):
    bf = ml_dtypes.bfloat16

    def to_sb_layout(W):  # [E, cols] -> [128, ECH*cols] partition-major
        cols = W.shape[1]
        return np.ascontiguousarray(
            W.reshape(ECH, 128, cols).transpose(1, 0, 2).reshape(128, ECH * cols)
        ).astype(bf)

    in_maps = []
    for c in range(NCORES):
        b, g = c // 4, c % 4
        xt = np.ascontiguousarray(np.asarray(x[b]).T).astype(bf).reshape(
            ECH, 128, T
        )
        wq = np.stack(
            [
                to_sb_layout(np.asarray(Wq[:, g * 512 + j * D: g * 512 + (j + 1) * D]))
                for j in range(HPG)
            ],
            axis=0,
        ).reshape(HPG, 128, ECH * D)
        wk = to_sb_layout(np.asarray(Wk[:, g * D:(g + 1) * D]))
        wv = to_sb_layout(np.asarray(Wv[:, g * D:(g + 1) * D]))
        wo = np.ascontiguousarray(Wo[g * 512:(g + 1) * 512, :]).astype(bf).reshape(
            HPG, 128, EMBED
        )
        bqc = np.ascontiguousarray(
            bq[g * 512:(g + 1) * 512].reshape(HPG, 128).T
        ).astype(np.float32)
        bkc = np.asarray(bk[g * D:(g + 1) * D]).reshape(128, 1).astype(np.float32)
        bvc = np.asarray(bv[g * D:(g + 1) * D]).reshape(1, 128).astype(bf)
        in_maps.append(
            {
                "xt": xt,
                "wq": wq,
                "wk": wk,
                "wv": wv,
                "wo": wo,
                "bq": bqc,
                "bk": bkc,
                "bv": bvc,
            }
        )
    return in_maps


def combine_outputs(results, bo):
    out = np.empty((2, T, EMBED), dtype=np.float32)
    for b in range(2):
        acc = results[b * 4]["out"].astype(np.float32)
        for g in range(1, 4):
            acc += results[b * 4 + g]["out"].astype(np.float32)
        out[b] = acc + np.asarray(bo)[None, :].astype(np.float32)
    return out


def kernel(x, Wq, bq, Wk, bk, Wv, bv, Wo, bo):
    from concourse.bass_utils import run_bass_kernel_spmd

    nc = build_program()
    in_maps = prepare_in_maps(x, Wq, bq, Wk, bk, Wv, bv, Wo, bo)
    res = run_bass_kernel_spmd(nc, in_maps, list(range(NCORES)))
    return combine_outputs(res.results, np.asarray(bo))
